# revision 20
# baseline (speedup 1.0000x reference)
"""Trainium2 Bass kernel for ExemplarImageMatching.

Math (per batch b):
  ei  = relu(bn1(W_img @ x))            x = image[b] as [C, HW]
  A   = s2*(Wa @ ei)                    (bn2 scale folded; Wa = W_dr[:, :C])
  ee  = relu(W_ex @ ex_b^T + b_ex)
  D   = s2*(Wb @ ee) + (s2*b_dr + t2)   (bias folded into D columns)
  sim[n, f] = sum_c relu(A[c, f] + D[c, n])^2
  out = softmax(sim / 0.1, axis=f)

Sharding: data-parallel over B across the 8 cores (B == 8), one image per
core; the N loop runs on-core.

Key structure (vs the one-hot fp32 row-sum baseline):
 - GEMM1/GEMM2 are 3-term Karatsuba-style f32r (tf32) matmuls (measured
   end-to-end error ~5e-5; any 2-term variant is ~2e-2).  x is split hi/lo
   on host; ei is split on device (DVE tensor_scalar copy rounds to tf32,
   gpsimd subtract forms the residual).
 - The channel sum runs on the PE with the roles swapped: the squared tile
   sq [128c, 128f] is the STATIONARY operand and a 16-wide one-hot column
   set (zsel slice) is the MOVING operand, accumulating sim^T [128f, 16n]
   slices in PSUM over (n, cb).  The fp32 4-cycle/row penalty then applies
   to a free dim of 16 instead of 512, and the reduction stays exact fp32.
   Per chunk that is 256 tiny matmuls (~27ns each) into one [128, 128]
   PSUM tile, then 8 PE transposes (via identity) restore sim [16, f].
 - Elementwise work is statically balanced across DVE/ACT/Pool:
   relu-adds mostly on DVE (tensor_scalar add+max runs in the 2x_2p DVE
   perf mode: all-SBUF fp32 at half cycle), squares on ACT (Square) and
   Pool (tensor_mul), PSUM evacuations (ei relu, A copy, sim^T copy) on
   Pool which pays no PSUM access penalty.
 - The GEMM pipeline for chunk k+1 (DMA, GEMM1, ei relu, hi/lo split,
   GEMM2, A copy) is emitted at fixed points inside chunk k's n-loop so
   every engine queue stays busy; 4 PSUM banks are reused
   GEMM1(k)->GEMM2(k)->GEMM1(k+1) with A evacuated to SBUF each chunk.

Softmax: per-chunk partial maxima accumulate off the critical path.
Chunks 0..NCH-2 exponentiate against the provisional max M3 while the last
chunk is still computing; a scalar factor gamma = exp(10*(M3 - M)) corrects
their denominators, and per-chunk normalize+DMA pipelines the stores.
"""

from contextlib import ExitStack

import numpy as np

import concourse.bass as bass
import concourse.bacc as bacc
import concourse.tile as tile
from concourse import mybir
from concourse.bass_utils import run_bass_kernel_spmd

B, N, C, H, W = 8, 16, 256, 64, 64
HW = H * W
P = 128
CB = C // P            # channel blocks (2)
FT = 512               # matmul free-dim tile (one PSUM bank of fp32)
FC = 1024              # f-chunk for the big elementwise pass
NCH = HW // FC         # 4
NBLK = FC // P         # 8 f-blocks of 128 per chunk
EPS = 1e-5
INV_TEMP = 10.0

F32 = mybir.dt.float32
F32R = mybir.dt.float32r
AF = mybir.ActivationFunctionType
OP = mybir.AluOpType
AX = mybir.AxisListType.X

# Static engine schedule for the big pass, per n of each chunk.
# relu-add (2 ops of [128,1024]): 'd'=DVE (593ns, 2x mode), 'p'=Pool (853),
#   'a'=ACT (1038).  square (1 op of [128,2048]): 'a'=ACT (1892),
#   'p'=Pool (1707), 'd'=DVE (2193, no 2x for scalar_tensor_tensor).
# GPSIMD cannot touch PSUM on TRN2, so all PSUM evacuations go to ACT/DVE.
RELU_ENG = [("d", "d")] * 16
for _i in (3, 7, 11):
    RELU_ENG[_i] = ("d", "p")
for _i in (5, 13):
    RELU_ENG[_i] = ("d", "a")
SQ_ENG = ["a", "p", "a", "p", "a", "p", "a", "p",
          "a", "p", "p", "p", "a", "p", "p", "p"]
# Last chunk: no successor-chunk work is interleaved, so the balance shifts:
# ACT squares run early (it finishes with exp+norm), Pool takes the late ones.
SQ_ENG_LAST = ["a", "p", "a", "p", "a", "p", "a", "p",
               "a", "p", "p", "p", "s", "s", "s", "s"]
RELU_ENG_LAST = [("d", "d")] * 16
for _i in (1, 4, 7):
    RELU_ENG_LAST[_i] = ("d", "p")
for _i in (6,):
    RELU_ENG_LAST[_i] = ("d", "a")


def _build_nc():
    nc = bacc.Bacc()

    xh_d = nc.dram_tensor("xh", [C, HW], F32R, kind="ExternalInput")
    xl_d = nc.dram_tensor("xl", [C, HW], F32R, kind="ExternalInput")
    wimgTh_d = nc.dram_tensor("wimgTh", [C, C], F32R, kind="ExternalInput")
    wimgTl_d = nc.dram_tensor("wimgTl", [C, C], F32R, kind="ExternalInput")
    waTh_d = nc.dram_tensor("waTh", [C, C], F32R, kind="ExternalInput")
    waTl_d = nc.dram_tensor("waTl", [C, C], F32R, kind="ExternalInput")
    wexT_d = nc.dram_tensor("wexT", [C, C], F32, kind="ExternalInput")
    wbT_d = nc.dram_tensor("wbT", [C, C], F32, kind="ExternalInput")
    exT_d = nc.dram_tensor("exT", [C, N], F32, kind="ExternalInput")
    bei_d = nc.dram_tensor("bei", [P, CB], F32, kind="ExternalInput")
    bA_d = nc.dram_tensor("bA", [P, CB], F32, kind="ExternalInput")
    bex_d = nc.dram_tensor("bex", [P, CB], F32, kind="ExternalInput")
    ident_d = nc.dram_tensor("ident", [P, P], F32, kind="ExternalInput")
    out_d = nc.dram_tensor("out", [N, HW], F32, kind="ExternalOutput")

    with ExitStack() as ctx:
        tc = ctx.enter_context(tile.TileContext(nc))
        singles = ctx.enter_context(tc.tile_pool(name="singles", bufs=1))
        xpool = ctx.enter_context(tc.tile_pool(name="xpool", bufs=2))
        eipool = ctx.enter_context(tc.tile_pool(name="eipool", bufs=1))
        espool = ctx.enter_context(tc.tile_pool(name="espool", bufs=1))
        apool = ctx.enter_context(tc.tile_pool(name="apool", bufs=2))
        rpool = ctx.enter_context(tc.tile_pool(name="rpool", bufs=4))
        sqpool = ctx.enter_context(tc.tile_pool(name="sqpool", bufs=4))
        stspool = ctx.enter_context(tc.tile_pool(name="stspool", bufs=2))
        wpool = ctx.enter_context(tc.tile_pool(name="wps", bufs=1, space="PSUM"))
        stpool = ctx.enter_context(tc.tile_pool(name="stps", bufs=2, space="PSUM"))
        sim_pool = ctx.enter_context(tc.tile_pool(name="sim_ps", bufs=1, space="PSUM"))

        # ---- constants / weights -------------------------------------------------
        # DMA order is latency-critical: chunk-0 x and the GEMM1 weights first
        # (everything funnels through the single SP HWDGE queue), the
        # exemplar-branch weights and ident later.
        def load(dram_ap, shape, tag, dt=F32):
            t = singles.tile(shape, dt, tag=tag, name=tag)
            nc.sync.dma_start(t[:], dram_ap)
            return t

        rr = lambda d: d[:, :].rearrange("(cb p) o -> p cb o", p=P)

        # warmup scratch (memset before anything else; used to ramp the PE
        # clock while the first DMAs are in flight)
        scratch = singles.tile([P, FT], F32)
        nc.gpsimd.memset(scratch[:], 0.0)
        # Z[:, N-1] = 1, rest 0.  Z[:, N-1-n : 2N-1-n] is a [P, N] matrix whose
        # column n is all-ones.
        zsel = singles.tile([P, 2 * N - 1], F32)
        nc.vector.memset(zsel[:], 0.0)
        nc.vector.memset(zsel[:, N - 1:N], 1.0)

        wps = wpool.tile([P, FT], F32, tag="g00", name="warm_ps")
        for i in range(2):
            nc.tensor.matmul(wps[:N, :], zsel[:, :N], scratch[:],
                             start=(i == 0), stop=(i == 1), skip_group_check=True)

        def load_act(dram_ap, shape, tag, dt=F32):
            t = singles.tile(shape, dt, tag=tag, name=tag)
            nc.scalar.dma_start(t[:], dram_ap)
            return t

        wimgTh = load(rr(wimgTh_d), [P, CB, C], "wimgTh", F32R)
        wimgTl = load(rr(wimgTl_d), [P, CB, C], "wimgTl", F32R)
        # constants ride the (otherwise idle) ACT HWDGE queue so the SP queue
        # carries only the GEMM-critical stream
        bei = load_act(bei_d[:, :], [P, CB], "bei")
        bA = load_act(bA_d[:, :], [P, CB], "bA")

        # ---- pipelined GEMM stages (chunk fc), emitted inside chunk fc-1 ---------
        xh_r = xh_d[:, :].rearrange("(cb p) hw -> p cb hw", p=P)
        xl_r = xl_d[:, :].rearrange("(cb p) hw -> p cb hw", p=P)
        state = {}

        def emit_xdma(fc):
            f0 = fc * FC
            xh_t = xpool.tile([P, CB, FC], F32R, tag="xh", name=f"xh{fc}")
            xl_t = xpool.tile([P, CB, FC], F32R, tag="xl", name=f"xl{fc}")
            for t2 in range(2):
                sl_s = slice(t2 * FT, (t2 + 1) * FT)
                sl_d = slice(f0 + t2 * FT, f0 + (t2 + 1) * FT)
                for cb in range(CB):
                    nc.sync.dma_start(xh_t[:, cb, sl_s], xh_r[:, cb, sl_d])
                for cb in range(CB):
                    nc.sync.dma_start(xl_t[:, cb, sl_s], xl_r[:, cb, sl_d])
            state[("x", fc)] = (xh_t, xl_t)

        def emit_gemm1(fc):
            xh_t, xl_t = state.pop(("x", fc))
            ps1 = {}
            for t2 in range(2):
                for ob in range(CB):
                    psx = wpool.tile([P, FT], F32, tag=f"g{ob}{t2}", name=f"ps1_{fc}_{ob}{t2}")
                    ps1[(ob, t2)] = psx
                    terms = [(wimgTh, xh_t), (wimgTl, xh_t), (wimgTh, xl_t)]
                    nt = len(terms)
                    for ti, (wt, xt) in enumerate(terms):
                        for cb in range(CB):
                            nc.tensor.matmul(
                                psx[:],
                                wt[:, cb, ob * P:(ob + 1) * P],
                                xt[:, cb, t2 * FT:(t2 + 1) * FT],
                                start=(ti == 0 and cb == 0),
                                stop=(ti == nt - 1 and cb == CB - 1),
                                skip_group_check=True,
                            )
            state[("ps1", fc)] = ps1

        def emit_eirelu(fc):
            ps1 = state.pop(("ps1", fc))
            ei_t = eipool.tile([P, CB, FC], F32, tag="ei", name=f"ei{fc}")
            for t2 in range(2):
                for ob in range(CB):
                    nc.scalar.activation(
                        ei_t[:, ob, t2 * FT:(t2 + 1) * FT], ps1[(ob, t2)][:],
                        AF.Relu, bias=bei[:, ob:ob + 1])
            state[("ei", fc)] = ei_t

        def emit_split(fc):
            ei_t = state[("ei", fc)]
            eih_t = espool.tile([P, CB, FC], F32R, tag="eih", name=f"eih{fc}")
            eil_t = espool.tile([P, CB, FC], F32R, tag="eil", name=f"eil{fc}")
            for t2 in range(2):
                sl = slice(t2 * FT, (t2 + 1) * FT)
                nc.vector.tensor_scalar(eih_t[:, :, sl], ei_t[:, :, sl], 1.0,
                                        None, op0=OP.mult)
                nc.gpsimd.tensor_tensor(eil_t[:, :, sl], ei_t[:, :, sl],
                                        eih_t[:, :, sl], op=OP.subtract)
            state.pop(("ei", fc))
            state[("eihl", fc)] = (eih_t, eil_t)

        def emit_gemm2(fc):
            eih_t, eil_t = state.pop(("eihl", fc))
            ps2 = {}
            for t2 in range(2):
                for ob in range(CB):
                    psx = wpool.tile([P, FT], F32, tag=f"g{ob}{t2}", name=f"ps2_{fc}_{ob}{t2}")
                    ps2[(ob, t2)] = psx
                    terms = [(waTh, eih_t), (waTl, eih_t), (waTh, eil_t)]
                    nt = len(terms)
                    for ti, (wt, et) in enumerate(terms):
                        for cb in range(CB):
                            nc.tensor.matmul(
                                psx[:],
                                wt[:, cb, ob * P:(ob + 1) * P],
                                et[:, cb, t2 * FT:(t2 + 1) * FT],
                                start=(ti == 0 and cb == 0),
                                stop=(ti == nt - 1 and cb == CB - 1),
                                skip_group_check=True,
                            )
            state[("ps2", fc)] = ps2

        def emit_acopy(fc):
            ps2 = state.pop(("ps2", fc))
            A_t = apool.tile([P, CB, FC], F32, tag="A", name=f"A{fc}")
            for t2 in range(2):
                for ob in range(CB):
                    dst = A_t[:, ob, t2 * FT:(t2 + 1) * FT]
                    if t2 == 0:
                        nc.vector.tensor_scalar(dst, ps2[(ob, t2)][:], 1.0,
                                                None, op0=OP.mult)
                    else:
                        nc.scalar.copy(dst, ps2[(ob, t2)][:])
            state[("A", fc)] = A_t

        # ---- prologue: chunk 0 GEMM pipeline ------------------------------------
        emit_xdma(0)
        waTh = load(rr(waTh_d), [P, CB, C], "waTh", F32R)
        waTl = load(rr(waTl_d), [P, CB, C], "waTl", F32R)
        exT = load_act(exT_d[:, :].rearrange("(cb p) n -> p cb n", p=P), [P, CB, N], "exT")
        wexT = load_act(rr(wexT_d), [P, CB, C], "wexT")
        bex = load_act(bex_d[:, :], [P, CB], "bex")
        wbT = load_act(rr(wbT_d), [P, CB, C], "wbT")
        emit_gemm1(0)
        emit_eirelu(0)
        emit_split(0)

        # ---- exemplar branch: ee = relu(WexT.T @ exT + bex);
        #      Dt = WbT.T @ ee + bA (bn2 bias folded into every column) -------
        ee = singles.tile([P, CB, N], F32)
        eeps = wpool.tile([P, FT], F32, tag="g00", name="ee_ps")
        for ob in range(CB):
            for cb in range(CB):
                nc.tensor.matmul(
                    eeps[:, ob * N:ob * N + N],
                    wexT[:, cb, ob * P:(ob + 1) * P],
                    exT[:, cb, :],
                    start=(cb == 0 and ob == 0), stop=(cb == CB - 1 and ob == CB - 1),
                    skip_group_check=True,
                )
        for ob in range(CB):
            nc.scalar.activation(ee[:, ob, :], eeps[:, ob * N:ob * N + N],
                                 AF.Relu, bias=bex[:, ob:ob + 1])
        Dt = singles.tile([P, CB, N], F32)
        dps = wpool.tile([P, FT], F32, tag="g00", name="d_ps")
        for ob in range(CB):
            for eb in range(CB):
                nc.tensor.matmul(
                    dps[:, ob * N:ob * N + N],
                    wbT[:, eb, ob * P:(ob + 1) * P],
                    ee[:, eb, :],
                    start=(eb == 0 and ob == 0), stop=(eb == CB - 1 and ob == CB - 1),
                    skip_group_check=True,
                )
        for ob in range(CB):
            nc.scalar.activation(Dt[:, ob, :], dps[:, ob * N:ob * N + N],
                                 AF.Identity, bias=bA[:, ob:ob + 1])

        emit_gemm2(0)
        emit_acopy(0)
        emit_xdma(1)
        ident = load_act(ident_d[:, :], [P, P], "ident")

        sim_sb = singles.tile([N, HW], F32)
        pmax = singles.tile([N, NCH], F32)   # running (cumulative) chunk maxima
        nmk = singles.tile([N, NCH], F32)    # -10 * running max per chunk
        dens = singles.tile([N, NCH], F32)   # per-chunk exp-sum vs its running max

        # ---- chunk loop ----------------------------------------------------------
        for fc in range(NCH):
            f0 = fc * FC
            A_t = state.pop(("A", fc))
            simT_ps = stpool.tile([P, P], F32, tag="simT", name=f"simT{fc}")
            nxt = fc + 1 if fc + 1 < NCH else None
            if nxt is not None:
                emit_gemm1(nxt)
            for n in range(N):
                r_t = rpool.tile([P, CB, FC], F32, tag="r", name=f"r{fc}_{n}")
                for cb in range(CB):
                    eng = (RELU_ENG[n] if fc < NCH - 1 else RELU_ENG_LAST[n])[cb]
                    if eng == "d":
                        nc.vector.tensor_scalar(
                            r_t[:, cb, :], A_t[:, cb, :], Dt[:, cb, n:n + 1],
                            0.0, op0=OP.add, op1=OP.max)
                    elif eng == "p":
                        nc.gpsimd.tensor_scalar(
                            r_t[:, cb, :], A_t[:, cb, :], Dt[:, cb, n:n + 1],
                            0.0, op0=OP.add, op1=OP.max)
                    else:
                        nc.scalar.activation(
                            r_t[:, cb, :], A_t[:, cb, :], AF.Relu,
                            bias=Dt[:, cb, n:n + 1])
                sq_t = sqpool.tile([P, CB, FC], F32, tag="sq", name=f"sq{fc}_{n}")
                seng = SQ_ENG[n] if fc < NCH - 1 else SQ_ENG_LAST[n]
                if seng == "a":
                    nc.scalar.activation(sq_t[:], r_t[:], AF.Square)
                elif seng == "p":
                    nc.gpsimd.tensor_mul(sq_t[:], r_t[:], r_t[:])
                elif seng == "s":
                    nc.scalar.activation(sq_t[:, 0, :], r_t[:, 0, :], AF.Square)
                    nc.gpsimd.tensor_mul(sq_t[:, 1, :], r_t[:, 1, :], r_t[:, 1, :])
                else:
                    nc.vector.scalar_tensor_tensor(
                        sq_t[:], r_t[:], 1.0, r_t[:], op0=OP.mult, op1=OP.mult)
                for cb in range(CB):
                    for b in range(NBLK):
                        nc.tensor.matmul(
                            simT_ps[:, b * N:(b + 1) * N],
                            sq_t[:, cb, b * P:(b + 1) * P],
                            zsel[:, N - 1 - n:2 * N - 1 - n],
                            start=(n == 0 and cb == 0 and b == 0),
                            stop=(n == N - 1 and cb == CB - 1 and b == NBLK - 1),
                            skip_group_check=True,
                        )
                # chunk fc+1 GEMM pipeline, interleaved into this chunk's queues
                if nxt is not None:
                    if n == 0 and nxt + 1 < NCH:
                        emit_xdma(nxt + 1)
                    elif n == 5:
                        emit_eirelu(nxt)
                    elif n == 7:
                        emit_split(nxt)
                    elif n == 9:
                        emit_gemm2(nxt)
                    elif n == 13:
                        emit_acopy(nxt)

            # evacuate sim^T, transpose back to [16, f], then chunk softmax piece:
            # exp against the RUNNING max M_fc = max(pmax[0..fc]); denominators
            # are rescaled at the end by gamma_fc = exp(10*(M_fc - M)) <= 1.
            simT_sb = stspool.tile([P, P], F32, tag="simTsb", name=f"simTsb{fc}")
            nc.vector.tensor_scalar(simT_sb[:], simT_ps[:], 1.0, None, op0=OP.mult)
            sim_ps = sim_pool.tile([N, FC], F32, tag="sim", name=f"sim_ps{fc}")
            if fc < NCH - 1:
                for b in range(NBLK):
                    nc.tensor.transpose(
                        sim_ps[:, b * P:(b + 1) * P], simT_sb[:, b * N:(b + 1) * N],
                        ident[:])
                nc.vector.reduce_max(pmax[:, fc:fc + 1], sim_ps[:], axis=AX)
            else:
                # split the max into halves pipelined with the transposes to
                # shorten the tail's serial chain
                pmh = singles.tile([N, 2], F32)
                for half in range(2):
                    for b in range(half * NBLK // 2, (half + 1) * NBLK // 2):
                        nc.tensor.transpose(
                            sim_ps[:, b * P:(b + 1) * P],
                            simT_sb[:, b * N:(b + 1) * N], ident[:])
                    nc.vector.reduce_max(
                        pmh[:, half:half + 1],
                        sim_ps[:, half * FC // 2:(half + 1) * FC // 2], axis=AX)
                nc.vector.reduce_max(pmax[:, fc:fc + 1], pmh[:], axis=AX)
            if fc > 0:
                nc.vector.tensor_tensor(pmax[:, fc:fc + 1], pmax[:, fc:fc + 1],
                                        pmax[:, fc - 1:fc], op=OP.max)
            nc.vector.tensor_scalar_mul(nmk[:, fc:fc + 1], pmax[:, fc:fc + 1],
                                        -INV_TEMP)
            nc.scalar.activation(
                sim_sb[:, f0:f0 + FC], sim_ps[:],
                AF.Exp, bias=nmk[:, fc:fc + 1], scale=INV_TEMP,
                accum_out=dens[:, fc:fc + 1],
            )

        # ---- softmax epilogue ----------------------------------------------------
        # M = pmax[:, NCH-1]; gamma_k = exp(10*(M_k - M)); den = sum_k gam_k*dens_k
        nmx = singles.tile([N, 1], F32)
        nc.vector.tensor_scalar_mul(nmx[:], pmax[:, NCH - 1:NCH], -INV_TEMP)
        gam = singles.tile([N, NCH], F32)
        nc.scalar.activation(gam[:], pmax[:], AF.Exp, bias=nmx[:], scale=INV_TEMP)
        gd = singles.tile([N, NCH], F32)
        nc.vector.tensor_mul(gd[:], gam[:], dens[:])
        den = singles.tile([N, 1], F32)
        nc.vector.reduce_sum(den[:], gd[:], axis=AX)
        rden = singles.tile([N, 1], F32)
        nc.vector.reciprocal(rden[:], den[:])
        grden = singles.tile([N, NCH], F32)
        nc.vector.tensor_scalar(grden[:], gam[:], rden[:, 0:1], None, op0=OP.mult)
        # normalize chunk k by gam_k/den; norms and stores spread across queues
        nc.scalar.activation(sim_sb[:, 3 * FC:], sim_sb[:, 3 * FC:], AF.Copy,
                             scale=grden[:, 3:4])
        nc.vector.tensor_scalar_mul(sim_sb[:, 0:FC], sim_sb[:, 0:FC], grden[:, 0:1])
        nc.sync.dma_start(out_d[:, 0:FC], sim_sb[:, 0:FC])
        nc.gpsimd.tensor_scalar(sim_sb[:, FC:2 * FC], sim_sb[:, FC:2 * FC],
                                grden[:, 1:2], None, op0=OP.mult)
        nc.scalar.dma_start(out_d[:, FC:2 * FC], sim_sb[:, FC:2 * FC])
        nc.vector.tensor_scalar_mul(sim_sb[:, 2 * FC:3 * FC], sim_sb[:, 2 * FC:3 * FC],
                                    grden[:, 2:3])
        nc.sync.dma_start(out_d[:, 2 * FC:3 * FC], sim_sb[:, 2 * FC:3 * FC])
        nc.scalar.dma_start(out_d[:, 3 * FC:], sim_sb[:, 3 * FC:])

    nc.compile()
    return nc


_NC_CACHE = {}


def _get_nc():
    if "nc" not in _NC_CACHE:
        _NC_CACHE["nc"] = _build_nc()
    return _NC_CACHE["nc"]


def _tf32(x):
    u = np.ascontiguousarray(x, dtype=np.float32).view(np.uint32)
    return ((u + np.uint32(0x1000)) & np.uint32(0xFFFFE000)).view(np.float32)


def _make_in_maps(inputs):
    f32 = np.float32
    img = np.ascontiguousarray(inputs["image_features"], dtype=f32)     # [B,C,H,W]
    ex = np.ascontiguousarray(inputs["exemplar_features"], dtype=f32)   # [B,N,C]

    s1 = (inputs["bn1_gamma"] / np.sqrt(inputs["bn1_var"] + EPS)).astype(f32)
    t1 = (inputs["bn1_beta"] - inputs["bn1_mean"] * s1).astype(f32)
    s2 = (inputs["bn2_gamma"] / np.sqrt(inputs["bn2_var"] + EPS)).astype(f32)
    t2 = (inputs["bn2_beta"] - inputs["bn2_mean"] * s2).astype(f32)

    W_img = np.asarray(inputs["W_img"], f32)
    W_dr = np.asarray(inputs["W_dr"], f32)
    W_ex = np.asarray(inputs["W_ex"], f32)

    wimg_f = s1[:, None] * W_img                       # [o, c]
    bei_full = (s1 * np.asarray(inputs["b_img"], f32) + t1).astype(f32)
    wa_f = s2[:, None] * W_dr[:, :C]
    bA_full = (s2 * np.asarray(inputs["b_dr"], f32) + t2).astype(f32)
    wb_f = s2[:, None] * W_dr[:, C:]
    bex_full = np.asarray(inputs["b_ex"], f32)

    def t(w):  # [o, c] -> [c, o], contiguous
        return np.ascontiguousarray(w.T.astype(f32))

    def pack_bias(v):  # [C] -> [P, CB], v[cb*P + p] at [p, cb]
        return np.ascontiguousarray(v.reshape(CB, P).T.astype(f32))

    def hl(w):  # hi/lo tf32 split
        h = _tf32(w)
        l = _tf32((w - h).astype(f32))
        return h, l

    wimgT = t(wimg_f)
    waT = t(wa_f)
    wimgTh, wimgTl = hl(wimgT)
    waTh, waTl = hl(waT)

    shared = {
        "wimgTh": wimgTh, "wimgTl": wimgTl,
        "waTh": waTh, "waTl": waTl,
        "wexT": t(W_ex),
        "wbT": t(wb_f),
        "bei": pack_bias(bei_full),
        "bA": pack_bias(bA_full),
        "bex": pack_bias(bex_full),
        "ident": np.eye(P, dtype=f32),
    }
    in_maps = []
    for b in range(B):
        m = dict(shared)
        x = np.ascontiguousarray(img[b].reshape(C, HW))
        xh = _tf32(x)
        xl = _tf32((x - xh).astype(f32))
        m["xh"] = xh
        m["xl"] = xl
        m["exT"] = np.ascontiguousarray(ex[b].T.astype(f32))
        in_maps.append(m)
    return in_maps


def _run(inputs, **kw):
    nc = _get_nc()
    in_maps = _make_in_maps(inputs)
    res = run_bass_kernel_spmd(nc, in_maps, core_ids=list(range(B)), **kw)
    out = np.stack([res.results[i]["out"] for i in range(B)])
    return out.reshape(B, N, H, W).astype(np.float32), res


def kernel(**inputs):
    out, _ = _run(inputs)
    return out


# revision 21
# speedup vs baseline: 1.0060x; 1.0060x over previous
"""Trainium2 Bass kernel for ExemplarImageMatching.

Math (per batch b):
  ei  = relu(bn1(W_img @ x))            x = image[b] as [C, HW]
  A   = s2*(Wa @ ei)                    (bn2 scale folded; Wa = W_dr[:, :C])
  ee  = relu(W_ex @ ex_b^T + b_ex)
  D   = s2*(Wb @ ee) + (s2*b_dr + t2)   (bias folded into D columns)
  sim[n, f] = sum_c relu(A[c, f] + D[c, n])^2
  out = softmax(sim / 0.1, axis=f)

Sharding: data-parallel over B across the 8 cores (B == 8), one image per
core; the N loop runs on-core.

Key structure (vs the one-hot fp32 row-sum baseline):
 - GEMM1/GEMM2 are 3-term Karatsuba-style f32r (tf32) matmuls (measured
   end-to-end error ~5e-5; any 2-term variant is ~2e-2).  x is split hi/lo
   on host; ei is split on device (DVE tensor_scalar copy rounds to tf32,
   gpsimd subtract forms the residual).
 - The channel sum runs on the PE with the roles swapped: the squared tile
   sq [128c, 128f] is the STATIONARY operand and a 16-wide one-hot column
   set (zsel slice) is the MOVING operand, accumulating sim^T [128f, 16n]
   slices in PSUM over (n, cb).  The fp32 4-cycle/row penalty then applies
   to a free dim of 16 instead of 512, and the reduction stays exact fp32.
   Per chunk that is 256 tiny matmuls (~27ns each) into one [128, 128]
   PSUM tile, then 8 PE transposes (via identity) restore sim [16, f].
 - Elementwise work is statically balanced across DVE/ACT/Pool:
   relu-adds mostly on DVE (tensor_scalar add+max runs in the 2x_2p DVE
   perf mode: all-SBUF fp32 at half cycle), squares on ACT (Square) and
   Pool (tensor_mul), PSUM evacuations (ei relu, A copy, sim^T copy) on
   Pool which pays no PSUM access penalty.
 - The GEMM pipeline for chunk k+1 (DMA, GEMM1, ei relu, hi/lo split,
   GEMM2, A copy) is emitted at fixed points inside chunk k's n-loop so
   every engine queue stays busy; 4 PSUM banks are reused
   GEMM1(k)->GEMM2(k)->GEMM1(k+1) with A evacuated to SBUF each chunk.

Softmax: per-chunk partial maxima accumulate off the critical path.
Chunks 0..NCH-2 exponentiate against the provisional max M3 while the last
chunk is still computing; a scalar factor gamma = exp(10*(M3 - M)) corrects
their denominators, and per-chunk normalize+DMA pipelines the stores.
"""

from contextlib import ExitStack

import numpy as np

import concourse.bass as bass
import concourse.bacc as bacc
import concourse.tile as tile
from concourse import mybir
from concourse.bass_utils import run_bass_kernel_spmd

B, N, C, H, W = 8, 16, 256, 64, 64
HW = H * W
P = 128
CB = C // P            # channel blocks (2)
FT = 512               # matmul free-dim tile (one PSUM bank of fp32)
FC = 1024              # f-chunk for the big elementwise pass
NCH = HW // FC         # 4
NBLK = FC // P         # 8 f-blocks of 128 per chunk
EPS = 1e-5
INV_TEMP = 10.0

F32 = mybir.dt.float32
F32R = mybir.dt.float32r
AF = mybir.ActivationFunctionType
OP = mybir.AluOpType
AX = mybir.AxisListType.X

# Static engine schedule for the big pass, per n of each chunk.
# relu-add (2 ops of [128,1024]): 'd'=DVE (593ns, 2x mode), 'p'=Pool (853),
#   'a'=ACT (1038).  square (1 op of [128,2048]): 'a'=ACT (1892),
#   'p'=Pool (1707), 'd'=DVE (2193, no 2x for scalar_tensor_tensor).
# GPSIMD cannot touch PSUM on TRN2, so all PSUM evacuations go to ACT/DVE.
RELU_ENG = [("d", "d")] * 16
for _i in (3, 7, 11, 15):
    RELU_ENG[_i] = ("d", "p")
for _i in (5, 13):
    RELU_ENG[_i] = ("d", "a")
SQ_ENG = ["a", "p", "a", "p", "a", "p", "a", "p",
          "a", "p", "a", "p", "a", "p", "p", "p"]
# Last chunk: no successor-chunk work is interleaved, so the balance shifts:
# ACT squares run early (it finishes with exp+norm), Pool takes the late ones.
SQ_ENG_LAST = ["a", "p", "a", "p", "a", "p", "a", "p",
               "a", "p", "p", "p", "s", "s", "s", "s"]
RELU_ENG_LAST = [("d", "d")] * 16
for _i in (1, 4, 7):
    RELU_ENG_LAST[_i] = ("d", "p")
for _i in (6,):
    RELU_ENG_LAST[_i] = ("d", "a")


def _build_nc():
    nc = bacc.Bacc()

    xh_d = nc.dram_tensor("xh", [C, HW], F32R, kind="ExternalInput")
    xl_d = nc.dram_tensor("xl", [C, HW], F32R, kind="ExternalInput")
    wimgTh_d = nc.dram_tensor("wimgTh", [C, C], F32R, kind="ExternalInput")
    wimgTl_d = nc.dram_tensor("wimgTl", [C, C], F32R, kind="ExternalInput")
    waTh_d = nc.dram_tensor("waTh", [C, C], F32R, kind="ExternalInput")
    waTl_d = nc.dram_tensor("waTl", [C, C], F32R, kind="ExternalInput")
    wexT_d = nc.dram_tensor("wexT", [C, C], F32, kind="ExternalInput")
    wbT_d = nc.dram_tensor("wbT", [C, C], F32, kind="ExternalInput")
    exT_d = nc.dram_tensor("exT", [C, N], F32, kind="ExternalInput")
    bei_d = nc.dram_tensor("bei", [P, CB], F32, kind="ExternalInput")
    bA_d = nc.dram_tensor("bA", [P, CB], F32, kind="ExternalInput")
    bex_d = nc.dram_tensor("bex", [P, CB], F32, kind="ExternalInput")
    ident_d = nc.dram_tensor("ident", [P, P], F32, kind="ExternalInput")
    out_d = nc.dram_tensor("out", [N, HW], F32, kind="ExternalOutput")

    with ExitStack() as ctx:
        tc = ctx.enter_context(tile.TileContext(nc))
        singles = ctx.enter_context(tc.tile_pool(name="singles", bufs=1))
        xpool = ctx.enter_context(tc.tile_pool(name="xpool", bufs=2))
        eipool = ctx.enter_context(tc.tile_pool(name="eipool", bufs=1))
        espool = ctx.enter_context(tc.tile_pool(name="espool", bufs=1))
        apool = ctx.enter_context(tc.tile_pool(name="apool", bufs=2))
        rpool = ctx.enter_context(tc.tile_pool(name="rpool", bufs=4))
        sqpool = ctx.enter_context(tc.tile_pool(name="sqpool", bufs=4))
        stspool = ctx.enter_context(tc.tile_pool(name="stspool", bufs=2))
        wpool = ctx.enter_context(tc.tile_pool(name="wps", bufs=1, space="PSUM"))
        stpool = ctx.enter_context(tc.tile_pool(name="stps", bufs=2, space="PSUM"))
        sim_pool = ctx.enter_context(tc.tile_pool(name="sim_ps", bufs=1, space="PSUM"))

        # ---- constants / weights -------------------------------------------------
        # DMA order is latency-critical: chunk-0 x and the GEMM1 weights first
        # (everything funnels through the single SP HWDGE queue), the
        # exemplar-branch weights and ident later.
        def load(dram_ap, shape, tag, dt=F32):
            t = singles.tile(shape, dt, tag=tag, name=tag)
            nc.sync.dma_start(t[:], dram_ap)
            return t

        rr = lambda d: d[:, :].rearrange("(cb p) o -> p cb o", p=P)

        # warmup scratch (memset before anything else; used to ramp the PE
        # clock while the first DMAs are in flight)
        scratch = singles.tile([P, FT], F32)
        nc.gpsimd.memset(scratch[:], 0.0)
        # Z[:, N-1] = 1, rest 0.  Z[:, N-1-n : 2N-1-n] is a [P, N] matrix whose
        # column n is all-ones.
        zsel = singles.tile([P, 2 * N - 1], F32)
        nc.vector.memset(zsel[:], 0.0)
        nc.vector.memset(zsel[:, N - 1:N], 1.0)

        wps = wpool.tile([P, FT], F32, tag="g00", name="warm_ps")
        for i in range(2):
            nc.tensor.matmul(wps[:N, :], zsel[:, :N], scratch[:],
                             start=(i == 0), stop=(i == 1), skip_group_check=True)

        def load_act(dram_ap, shape, tag, dt=F32):
            t = singles.tile(shape, dt, tag=tag, name=tag)
            nc.scalar.dma_start(t[:], dram_ap)
            return t

        wimgTh = load(rr(wimgTh_d), [P, CB, C], "wimgTh", F32R)
        wimgTl = load(rr(wimgTl_d), [P, CB, C], "wimgTl", F32R)
        # constants ride the (otherwise idle) ACT HWDGE queue so the SP queue
        # carries only the GEMM-critical stream
        bei = load_act(bei_d[:, :], [P, CB], "bei")
        bA = load_act(bA_d[:, :], [P, CB], "bA")

        # ---- pipelined GEMM stages (chunk fc), emitted inside chunk fc-1 ---------
        xh_r = xh_d[:, :].rearrange("(cb p) hw -> p cb hw", p=P)
        xl_r = xl_d[:, :].rearrange("(cb p) hw -> p cb hw", p=P)
        state = {}

        def emit_xdma(fc):
            f0 = fc * FC
            xh_t = xpool.tile([P, CB, FC], F32R, tag="xh", name=f"xh{fc}")
            xl_t = xpool.tile([P, CB, FC], F32R, tag="xl", name=f"xl{fc}")
            for t2 in range(2):
                sl_s = slice(t2 * FT, (t2 + 1) * FT)
                sl_d = slice(f0 + t2 * FT, f0 + (t2 + 1) * FT)
                for cb in range(CB):
                    nc.sync.dma_start(xh_t[:, cb, sl_s], xh_r[:, cb, sl_d])
                for cb in range(CB):
                    nc.sync.dma_start(xl_t[:, cb, sl_s], xl_r[:, cb, sl_d])
            state[("x", fc)] = (xh_t, xl_t)

        def emit_gemm1(fc):
            xh_t, xl_t = state.pop(("x", fc))
            ps1 = {}
            for t2 in range(2):
                for ob in range(CB):
                    psx = wpool.tile([P, FT], F32, tag=f"g{ob}{t2}", name=f"ps1_{fc}_{ob}{t2}")
                    ps1[(ob, t2)] = psx
                    terms = [(wimgTh, xh_t), (wimgTl, xh_t), (wimgTh, xl_t)]
                    nt = len(terms)
                    for ti, (wt, xt) in enumerate(terms):
                        for cb in range(CB):
                            nc.tensor.matmul(
                                psx[:],
                                wt[:, cb, ob * P:(ob + 1) * P],
                                xt[:, cb, t2 * FT:(t2 + 1) * FT],
                                start=(ti == 0 and cb == 0),
                                stop=(ti == nt - 1 and cb == CB - 1),
                                skip_group_check=True,
                            )
            state[("ps1", fc)] = ps1

        def emit_eirelu(fc):
            ps1 = state.pop(("ps1", fc))
            ei_t = eipool.tile([P, CB, FC], F32, tag="ei", name=f"ei{fc}")
            for t2 in range(2):
                for ob in range(CB):
                    nc.scalar.activation(
                        ei_t[:, ob, t2 * FT:(t2 + 1) * FT], ps1[(ob, t2)][:],
                        AF.Relu, bias=bei[:, ob:ob + 1])
            state[("ei", fc)] = ei_t

        def emit_split(fc):
            ei_t = state[("ei", fc)]
            eih_t = espool.tile([P, CB, FC], F32R, tag="eih", name=f"eih{fc}")
            eil_t = espool.tile([P, CB, FC], F32R, tag="eil", name=f"eil{fc}")
            for t2 in range(2):
                sl = slice(t2 * FT, (t2 + 1) * FT)
                nc.vector.tensor_scalar(eih_t[:, :, sl], ei_t[:, :, sl], 1.0,
                                        None, op0=OP.mult)
                nc.gpsimd.tensor_tensor(eil_t[:, :, sl], ei_t[:, :, sl],
                                        eih_t[:, :, sl], op=OP.subtract)
            state.pop(("ei", fc))
            state[("eihl", fc)] = (eih_t, eil_t)

        def emit_gemm2(fc):
            eih_t, eil_t = state.pop(("eihl", fc))
            ps2 = {}
            for t2 in range(2):
                for ob in range(CB):
                    psx = wpool.tile([P, FT], F32, tag=f"g{ob}{t2}", name=f"ps2_{fc}_{ob}{t2}")
                    ps2[(ob, t2)] = psx
                    terms = [(waTh, eih_t), (waTl, eih_t), (waTh, eil_t)]
                    nt = len(terms)
                    for ti, (wt, et) in enumerate(terms):
                        for cb in range(CB):
                            nc.tensor.matmul(
                                psx[:],
                                wt[:, cb, ob * P:(ob + 1) * P],
                                et[:, cb, t2 * FT:(t2 + 1) * FT],
                                start=(ti == 0 and cb == 0),
                                stop=(ti == nt - 1 and cb == CB - 1),
                                skip_group_check=True,
                            )
            state[("ps2", fc)] = ps2

        def emit_acopy(fc):
            ps2 = state.pop(("ps2", fc))
            A_t = apool.tile([P, CB, FC], F32, tag="A", name=f"A{fc}")
            for t2 in range(2):
                for ob in range(CB):
                    dst = A_t[:, ob, t2 * FT:(t2 + 1) * FT]
                    nc.vector.tensor_scalar(dst, ps2[(ob, t2)][:], 1.0,
                                            None, op0=OP.mult)
            state[("A", fc)] = A_t

        # ---- prologue: chunk 0 GEMM pipeline ------------------------------------
        emit_xdma(0)
        waTh = load(rr(waTh_d), [P, CB, C], "waTh", F32R)
        waTl = load(rr(waTl_d), [P, CB, C], "waTl", F32R)
        exT = load_act(exT_d[:, :].rearrange("(cb p) n -> p cb n", p=P), [P, CB, N], "exT")
        wexT = load_act(rr(wexT_d), [P, CB, C], "wexT")
        bex = load_act(bex_d[:, :], [P, CB], "bex")
        wbT = load_act(rr(wbT_d), [P, CB, C], "wbT")
        emit_gemm1(0)
        emit_eirelu(0)
        emit_split(0)

        # ---- exemplar branch: ee = relu(WexT.T @ exT + bex);
        #      Dt = WbT.T @ ee + bA (bn2 bias folded into every column) -------
        ee = singles.tile([P, CB, N], F32)
        eeps = wpool.tile([P, FT], F32, tag="g00", name="ee_ps")
        for ob in range(CB):
            for cb in range(CB):
                nc.tensor.matmul(
                    eeps[:, ob * N:ob * N + N],
                    wexT[:, cb, ob * P:(ob + 1) * P],
                    exT[:, cb, :],
                    start=(cb == 0 and ob == 0), stop=(cb == CB - 1 and ob == CB - 1),
                    skip_group_check=True,
                )
        for ob in range(CB):
            nc.scalar.activation(ee[:, ob, :], eeps[:, ob * N:ob * N + N],
                                 AF.Relu, bias=bex[:, ob:ob + 1])
        Dt = singles.tile([P, CB, N], F32)
        dps = wpool.tile([P, FT], F32, tag="g00", name="d_ps")
        for ob in range(CB):
            for eb in range(CB):
                nc.tensor.matmul(
                    dps[:, ob * N:ob * N + N],
                    wbT[:, eb, ob * P:(ob + 1) * P],
                    ee[:, eb, :],
                    start=(eb == 0 and ob == 0), stop=(eb == CB - 1 and ob == CB - 1),
                    skip_group_check=True,
                )
        for ob in range(CB):
            nc.scalar.activation(Dt[:, ob, :], dps[:, ob * N:ob * N + N],
                                 AF.Identity, bias=bA[:, ob:ob + 1])

        emit_gemm2(0)
        emit_acopy(0)
        emit_xdma(1)
        ident = load(ident_d[:, :], [P, P], "ident")

        sim_sb = singles.tile([N, HW], F32)
        pmax = singles.tile([N, NCH], F32)   # running (cumulative) chunk maxima
        nmk = singles.tile([N, NCH], F32)    # -10 * running max per chunk
        dens = singles.tile([N, NCH], F32)   # per-chunk exp-sum vs its running max

        # ---- chunk loop ----------------------------------------------------------
        for fc in range(NCH):
            f0 = fc * FC
            A_t = state.pop(("A", fc))
            simT_ps = stpool.tile([P, P], F32, tag="simT", name=f"simT{fc}")
            nxt = fc + 1 if fc + 1 < NCH else None
            if nxt is not None:
                emit_gemm1(nxt)
            for n in range(N):
                r_t = rpool.tile([P, CB, FC], F32, tag="r", name=f"r{fc}_{n}")
                for cb in range(CB):
                    eng = (RELU_ENG[n] if fc < NCH - 1 else RELU_ENG_LAST[n])[cb]
                    if eng == "d":
                        nc.vector.tensor_scalar(
                            r_t[:, cb, :], A_t[:, cb, :], Dt[:, cb, n:n + 1],
                            0.0, op0=OP.add, op1=OP.max)
                    elif eng == "p":
                        nc.gpsimd.tensor_scalar(
                            r_t[:, cb, :], A_t[:, cb, :], Dt[:, cb, n:n + 1],
                            0.0, op0=OP.add, op1=OP.max)
                    else:
                        nc.scalar.activation(
                            r_t[:, cb, :], A_t[:, cb, :], AF.Relu,
                            bias=Dt[:, cb, n:n + 1])
                sq_t = sqpool.tile([P, CB, FC], F32, tag="sq", name=f"sq{fc}_{n}")
                seng = SQ_ENG[n] if fc < NCH - 1 else SQ_ENG_LAST[n]
                if seng == "a":
                    nc.scalar.activation(sq_t[:], r_t[:], AF.Square)
                elif seng == "p":
                    nc.gpsimd.tensor_mul(sq_t[:], r_t[:], r_t[:])
                elif seng == "s":
                    nc.scalar.activation(sq_t[:, 0, :], r_t[:, 0, :], AF.Square)
                    nc.gpsimd.tensor_mul(sq_t[:, 1, :], r_t[:, 1, :], r_t[:, 1, :])
                else:
                    nc.vector.scalar_tensor_tensor(
                        sq_t[:], r_t[:], 1.0, r_t[:], op0=OP.mult, op1=OP.mult)
                for cb in range(CB):
                    for b in range(NBLK):
                        nc.tensor.matmul(
                            simT_ps[:, b * N:(b + 1) * N],
                            sq_t[:, cb, b * P:(b + 1) * P],
                            zsel[:, N - 1 - n:2 * N - 1 - n],
                            start=(n == 0 and cb == 0 and b == 0),
                            stop=(n == N - 1 and cb == CB - 1 and b == NBLK - 1),
                            skip_group_check=True,
                        )
                # chunk fc+1 GEMM pipeline, interleaved into this chunk's queues
                if nxt is not None:
                    if n == 0 and nxt + 1 < NCH:
                        emit_xdma(nxt + 1)
                    elif n == 5:
                        emit_eirelu(nxt)
                    elif n == 7:
                        emit_split(nxt)
                    elif n == 9:
                        emit_gemm2(nxt)
                    elif n == 13:
                        emit_acopy(nxt)

            # evacuate sim^T, transpose back to [16, f], then chunk softmax piece:
            # exp against the RUNNING max M_fc = max(pmax[0..fc]); denominators
            # are rescaled at the end by gamma_fc = exp(10*(M_fc - M)) <= 1.
            simT_sb = stspool.tile([P, P], F32, tag="simTsb", name=f"simTsb{fc}")
            nc.vector.tensor_scalar(simT_sb[:], simT_ps[:], 1.0, None, op0=OP.mult)
            sim_ps = sim_pool.tile([N, FC], F32, tag="sim", name=f"sim_ps{fc}")
            if fc < NCH - 1:
                for b in range(NBLK):
                    nc.tensor.transpose(
                        sim_ps[:, b * P:(b + 1) * P], simT_sb[:, b * N:(b + 1) * N],
                        ident[:])
                nc.vector.reduce_max(pmax[:, fc:fc + 1], sim_ps[:], axis=AX)
            else:
                # split the max into halves pipelined with the transposes to
                # shorten the tail's serial chain
                pmh = singles.tile([N, 2], F32)
                for half in range(2):
                    for b in range(half * NBLK // 2, (half + 1) * NBLK // 2):
                        nc.tensor.transpose(
                            sim_ps[:, b * P:(b + 1) * P],
                            simT_sb[:, b * N:(b + 1) * N], ident[:])
                    nc.vector.reduce_max(
                        pmh[:, half:half + 1],
                        sim_ps[:, half * FC // 2:(half + 1) * FC // 2], axis=AX)
                nc.vector.reduce_max(pmax[:, fc:fc + 1], pmh[:], axis=AX)
            if fc > 0:
                nc.vector.tensor_tensor(pmax[:, fc:fc + 1], pmax[:, fc:fc + 1],
                                        pmax[:, fc - 1:fc], op=OP.max)
            nc.vector.tensor_scalar_mul(nmk[:, fc:fc + 1], pmax[:, fc:fc + 1],
                                        -INV_TEMP)
            nc.scalar.activation(
                sim_sb[:, f0:f0 + FC], sim_ps[:],
                AF.Exp, bias=nmk[:, fc:fc + 1], scale=INV_TEMP,
                accum_out=dens[:, fc:fc + 1],
            )

        # ---- softmax epilogue ----------------------------------------------------
        # M = pmax[:, NCH-1]; gamma_k = exp(10*(M_k - M)); den = sum_k gam_k*dens_k
        nmx = singles.tile([N, 1], F32)
        nc.vector.tensor_scalar_mul(nmx[:], pmax[:, NCH - 1:NCH], -INV_TEMP)
        gam = singles.tile([N, NCH], F32)
        nc.scalar.activation(gam[:], pmax[:], AF.Exp, bias=nmx[:], scale=INV_TEMP)
        gd = singles.tile([N, NCH], F32)
        nc.vector.tensor_mul(gd[:], gam[:], dens[:])
        den = singles.tile([N, 1], F32)
        nc.vector.reduce_sum(den[:], gd[:], axis=AX)
        rden = singles.tile([N, 1], F32)
        nc.vector.reciprocal(rden[:], den[:])
        grden = singles.tile([N, NCH], F32)
        nc.vector.tensor_scalar(grden[:], gam[:], rden[:, 0:1], None, op0=OP.mult)
        # normalize chunk k by gam_k/den; norms and stores spread across queues
        nc.scalar.activation(sim_sb[:, 3 * FC:], sim_sb[:, 3 * FC:], AF.Copy,
                             scale=grden[:, 3:4])
        nc.vector.tensor_scalar_mul(sim_sb[:, 0:FC], sim_sb[:, 0:FC], grden[:, 0:1])
        nc.sync.dma_start(out_d[:, 0:FC], sim_sb[:, 0:FC])
        nc.gpsimd.tensor_scalar(sim_sb[:, FC:2 * FC], sim_sb[:, FC:2 * FC],
                                grden[:, 1:2], None, op0=OP.mult)
        nc.scalar.dma_start(out_d[:, FC:2 * FC], sim_sb[:, FC:2 * FC])
        nc.vector.tensor_scalar_mul(sim_sb[:, 2 * FC:3 * FC], sim_sb[:, 2 * FC:3 * FC],
                                    grden[:, 2:3])
        nc.sync.dma_start(out_d[:, 2 * FC:3 * FC], sim_sb[:, 2 * FC:3 * FC])
        nc.scalar.dma_start(out_d[:, 3 * FC:], sim_sb[:, 3 * FC:])

    nc.compile()
    return nc


_NC_CACHE = {}


def _get_nc():
    if "nc" not in _NC_CACHE:
        _NC_CACHE["nc"] = _build_nc()
    return _NC_CACHE["nc"]


def _tf32(x):
    u = np.ascontiguousarray(x, dtype=np.float32).view(np.uint32)
    return ((u + np.uint32(0x1000)) & np.uint32(0xFFFFE000)).view(np.float32)


def _make_in_maps(inputs):
    f32 = np.float32
    img = np.ascontiguousarray(inputs["image_features"], dtype=f32)     # [B,C,H,W]
    ex = np.ascontiguousarray(inputs["exemplar_features"], dtype=f32)   # [B,N,C]

    s1 = (inputs["bn1_gamma"] / np.sqrt(inputs["bn1_var"] + EPS)).astype(f32)
    t1 = (inputs["bn1_beta"] - inputs["bn1_mean"] * s1).astype(f32)
    s2 = (inputs["bn2_gamma"] / np.sqrt(inputs["bn2_var"] + EPS)).astype(f32)
    t2 = (inputs["bn2_beta"] - inputs["bn2_mean"] * s2).astype(f32)

    W_img = np.asarray(inputs["W_img"], f32)
    W_dr = np.asarray(inputs["W_dr"], f32)
    W_ex = np.asarray(inputs["W_ex"], f32)

    wimg_f = s1[:, None] * W_img                       # [o, c]
    bei_full = (s1 * np.asarray(inputs["b_img"], f32) + t1).astype(f32)
    wa_f = s2[:, None] * W_dr[:, :C]
    bA_full = (s2 * np.asarray(inputs["b_dr"], f32) + t2).astype(f32)
    wb_f = s2[:, None] * W_dr[:, C:]
    bex_full = np.asarray(inputs["b_ex"], f32)

    def t(w):  # [o, c] -> [c, o], contiguous
        return np.ascontiguousarray(w.T.astype(f32))

    def pack_bias(v):  # [C] -> [P, CB], v[cb*P + p] at [p, cb]
        return np.ascontiguousarray(v.reshape(CB, P).T.astype(f32))

    def hl(w):  # hi/lo tf32 split
        h = _tf32(w)
        l = _tf32((w - h).astype(f32))
        return h, l

    wimgT = t(wimg_f)
    waT = t(wa_f)
    wimgTh, wimgTl = hl(wimgT)
    waTh, waTl = hl(waT)

    shared = {
        "wimgTh": wimgTh, "wimgTl": wimgTl,
        "waTh": waTh, "waTl": waTl,
        "wexT": t(W_ex),
        "wbT": t(wb_f),
        "bei": pack_bias(bei_full),
        "bA": pack_bias(bA_full),
        "bex": pack_bias(bex_full),
        "ident": np.eye(P, dtype=f32),
    }
    in_maps = []
    for b in range(B):
        m = dict(shared)
        x = np.ascontiguousarray(img[b].reshape(C, HW))
        xh = _tf32(x)
        xl = _tf32((x - xh).astype(f32))
        m["xh"] = xh
        m["xl"] = xl
        m["exT"] = np.ascontiguousarray(ex[b].T.astype(f32))
        in_maps.append(m)
    return in_maps


def _run(inputs, **kw):
    nc = _get_nc()
    in_maps = _make_in_maps(inputs)
    res = run_bass_kernel_spmd(nc, in_maps, core_ids=list(range(B)), **kw)
    out = np.stack([res.results[i]["out"] for i in range(B)])
    return out.reshape(B, N, H, W).astype(np.float32), res


def kernel(**inputs):
    out, _ = _run(inputs)
    return out


# revision 23
# speedup vs baseline: 1.0067x; 1.0008x over previous
"""Trainium2 Bass kernel for ExemplarImageMatching.

Math (per batch b):
  ei  = relu(bn1(W_img @ x))            x = image[b] as [C, HW]
  A   = s2*(Wa @ ei)                    (bn2 scale folded; Wa = W_dr[:, :C])
  ee  = relu(W_ex @ ex_b^T + b_ex)
  D   = s2*(Wb @ ee) + (s2*b_dr + t2)   (bias folded into D columns)
  sim[n, f] = sum_c relu(A[c, f] + D[c, n])^2
  out = softmax(sim / 0.1, axis=f)

Sharding: data-parallel over B across the 8 cores (B == 8), one image per
core; the N loop runs on-core.

Key structure (vs the one-hot fp32 row-sum baseline):
 - GEMM1/GEMM2 are 3-term Karatsuba-style f32r (tf32) matmuls (measured
   end-to-end error ~5e-5; any 2-term variant is ~2e-2).  x is split hi/lo
   on host; ei is split on device (DVE tensor_scalar copy rounds to tf32,
   gpsimd subtract forms the residual).
 - The channel sum runs on the PE with the roles swapped: the squared tile
   sq [128c, 128f] is the STATIONARY operand and a 16-wide one-hot column
   set (zsel slice) is the MOVING operand, accumulating sim^T [128f, 16n]
   slices in PSUM over (n, cb).  The fp32 4-cycle/row penalty then applies
   to a free dim of 16 instead of 512, and the reduction stays exact fp32.
   Per chunk that is 256 tiny matmuls (~27ns each) into one [128, 128]
   PSUM tile, then 8 PE transposes (via identity) restore sim [16, f].
 - Elementwise work is statically balanced across DVE/ACT/Pool:
   relu-adds mostly on DVE (tensor_scalar add+max runs in the 2x_2p DVE
   perf mode: all-SBUF fp32 at half cycle), squares on ACT (Square) and
   Pool (tensor_mul), PSUM evacuations (ei relu, A copy, sim^T copy) on
   Pool which pays no PSUM access penalty.
 - The GEMM pipeline for chunk k+1 (DMA, GEMM1, ei relu, hi/lo split,
   GEMM2, A copy) is emitted at fixed points inside chunk k's n-loop so
   every engine queue stays busy; 4 PSUM banks are reused
   GEMM1(k)->GEMM2(k)->GEMM1(k+1) with A evacuated to SBUF each chunk.

Softmax: per-chunk partial maxima accumulate off the critical path.
Chunks 0..NCH-2 exponentiate against the provisional max M3 while the last
chunk is still computing; a scalar factor gamma = exp(10*(M3 - M)) corrects
their denominators, and per-chunk normalize+DMA pipelines the stores.
"""

from contextlib import ExitStack

import numpy as np

import concourse.bass as bass
import concourse.bacc as bacc
import concourse.tile as tile
from concourse import mybir
from concourse.bass_utils import run_bass_kernel_spmd

B, N, C, H, W = 8, 16, 256, 64, 64
HW = H * W
P = 128
CB = C // P            # channel blocks (2)
FT = 512               # matmul free-dim tile (one PSUM bank of fp32)
FC = 1024              # f-chunk for the big elementwise pass
NCH = HW // FC         # 4
NBLK = FC // P         # 8 f-blocks of 128 per chunk
EPS = 1e-5
INV_TEMP = 10.0

F32 = mybir.dt.float32
F32R = mybir.dt.float32r
AF = mybir.ActivationFunctionType
OP = mybir.AluOpType
AX = mybir.AxisListType.X

# Static engine schedule for the big pass, per n of each chunk.
# relu-add (2 ops of [128,1024]): 'd'=DVE (593ns, 2x mode), 'p'=Pool (853),
#   'a'=ACT (1038).  square (1 op of [128,2048]): 'a'=ACT (1892),
#   'p'=Pool (1707), 'd'=DVE (2193, no 2x for scalar_tensor_tensor).
# GPSIMD cannot touch PSUM on TRN2, so all PSUM evacuations go to ACT/DVE.
RELU_ENG = [("d", "d")] * 16
for _i in (3, 7, 11, 15):
    RELU_ENG[_i] = ("d", "p")
for _i in (5, 13):
    RELU_ENG[_i] = ("d", "a")
SQ_ENG = ["a", "p", "a", "p", "a", "p", "a", "p",
          "a", "p", "a", "p", "a", "p", "p", "p"]
# Last chunk: no successor-chunk work is interleaved, so the balance shifts:
# ACT squares run early (it finishes with exp+norm), Pool takes the late ones.
SQ_ENG_LAST = ["a", "p", "a", "p", "a", "p", "a", "p",
               "a", "p", "p", "p", "s", "s", "s", "s"]
RELU_ENG_LAST = [("d", "d")] * 16
for _i in (1, 4, 7):
    RELU_ENG_LAST[_i] = ("d", "p")
for _i in (6,):
    RELU_ENG_LAST[_i] = ("d", "a")


def _build_nc():
    nc = bacc.Bacc()

    xh_d = nc.dram_tensor("xh", [C, HW], F32R, kind="ExternalInput")
    xl_d = nc.dram_tensor("xl", [C, HW], F32R, kind="ExternalInput")
    wimgTh_d = nc.dram_tensor("wimgTh", [C, C], F32R, kind="ExternalInput")
    wimgTl_d = nc.dram_tensor("wimgTl", [C, C], F32R, kind="ExternalInput")
    waTh_d = nc.dram_tensor("waTh", [C, C], F32R, kind="ExternalInput")
    waTl_d = nc.dram_tensor("waTl", [C, C], F32R, kind="ExternalInput")
    wexT_d = nc.dram_tensor("wexT", [C, C], F32, kind="ExternalInput")
    wbT_d = nc.dram_tensor("wbT", [C, C], F32, kind="ExternalInput")
    exT_d = nc.dram_tensor("exT", [C, N], F32, kind="ExternalInput")
    bei_d = nc.dram_tensor("bei", [P, CB], F32, kind="ExternalInput")
    bA_d = nc.dram_tensor("bA", [P, CB], F32, kind="ExternalInput")
    bex_d = nc.dram_tensor("bex", [P, CB], F32, kind="ExternalInput")
    ident_d = nc.dram_tensor("ident", [P, P], F32, kind="ExternalInput")
    out_d = nc.dram_tensor("out", [N, HW], F32, kind="ExternalOutput")

    with ExitStack() as ctx:
        tc = ctx.enter_context(tile.TileContext(nc))
        singles = ctx.enter_context(tc.tile_pool(name="singles", bufs=1))
        xpool = ctx.enter_context(tc.tile_pool(name="xpool", bufs=2))
        eipool = ctx.enter_context(tc.tile_pool(name="eipool", bufs=1))
        espool = ctx.enter_context(tc.tile_pool(name="espool", bufs=1))
        apool = ctx.enter_context(tc.tile_pool(name="apool", bufs=2))
        rpool = ctx.enter_context(tc.tile_pool(name="rpool", bufs=4))
        sqpool = ctx.enter_context(tc.tile_pool(name="sqpool", bufs=4))
        stspool = ctx.enter_context(tc.tile_pool(name="stspool", bufs=2))
        wpool = ctx.enter_context(tc.tile_pool(name="wps", bufs=1, space="PSUM"))
        stpool = ctx.enter_context(tc.tile_pool(name="stps", bufs=2, space="PSUM"))
        sim_pool = ctx.enter_context(tc.tile_pool(name="sim_ps", bufs=1, space="PSUM"))

        # ---- constants / weights -------------------------------------------------
        # DMA order is latency-critical: chunk-0 x and the GEMM1 weights first
        # (everything funnels through the single SP HWDGE queue), the
        # exemplar-branch weights and ident later.
        def load(dram_ap, shape, tag, dt=F32):
            t = singles.tile(shape, dt, tag=tag, name=tag)
            nc.sync.dma_start(t[:], dram_ap)
            return t

        rr = lambda d: d[:, :].rearrange("(cb p) o -> p cb o", p=P)

        # warmup scratch (memset before anything else; used to ramp the PE
        # clock while the first DMAs are in flight)
        scratch = singles.tile([P, FT], F32)
        nc.gpsimd.memset(scratch[:], 0.0)
        # Z[:, N-1] = 1, rest 0.  Z[:, N-1-n : 2N-1-n] is a [P, N] matrix whose
        # column n is all-ones.
        zsel = singles.tile([P, 2 * N - 1], F32)
        nc.vector.memset(zsel[:], 0.0)
        nc.vector.memset(zsel[:, N - 1:N], 1.0)

        wps = wpool.tile([P, FT], F32, tag="g00", name="warm_ps")
        for i in range(2):
            nc.tensor.matmul(wps[:N, :], zsel[:, :N], scratch[:],
                             start=(i == 0), stop=(i == 1), skip_group_check=True)

        def load_act(dram_ap, shape, tag, dt=F32):
            t = singles.tile(shape, dt, tag=tag, name=tag)
            nc.scalar.dma_start(t[:], dram_ap)
            return t

        wimgTh = load(rr(wimgTh_d), [P, CB, C], "wimgTh", F32R)
        wimgTl = load(rr(wimgTl_d), [P, CB, C], "wimgTl", F32R)
        # constants ride the (otherwise idle) ACT HWDGE queue so the SP queue
        # carries only the GEMM-critical stream
        bei = load_act(bei_d[:, :], [P, CB], "bei")
        bA = load_act(bA_d[:, :], [P, CB], "bA")

        # ---- pipelined GEMM stages (chunk fc), emitted inside chunk fc-1 ---------
        xh_r = xh_d[:, :].rearrange("(cb p) hw -> p cb hw", p=P)
        xl_r = xl_d[:, :].rearrange("(cb p) hw -> p cb hw", p=P)
        state = {}

        def emit_xdma(fc):
            f0 = fc * FC
            xh_t = xpool.tile([P, CB, FC], F32R, tag="xh", name=f"xh{fc}")
            xl_t = xpool.tile([P, CB, FC], F32R, tag="xl", name=f"xl{fc}")
            for t2 in range(2):
                sl_s = slice(t2 * FT, (t2 + 1) * FT)
                sl_d = slice(f0 + t2 * FT, f0 + (t2 + 1) * FT)
                for cb in range(CB):
                    nc.sync.dma_start(xh_t[:, cb, sl_s], xh_r[:, cb, sl_d])
                for cb in range(CB):
                    nc.sync.dma_start(xl_t[:, cb, sl_s], xl_r[:, cb, sl_d])
            state[("x", fc)] = (xh_t, xl_t)

        def emit_gemm1(fc):
            xh_t, xl_t = state.pop(("x", fc))
            ps1 = {}
            for t2 in range(2):
                for ob in range(CB):
                    psx = wpool.tile([P, FT], F32, tag=f"g{ob}{t2}", name=f"ps1_{fc}_{ob}{t2}")
                    ps1[(ob, t2)] = psx
                    terms = [(wimgTh, xh_t), (wimgTl, xh_t), (wimgTh, xl_t)]
                    nt = len(terms)
                    for ti, (wt, xt) in enumerate(terms):
                        for cb in range(CB):
                            nc.tensor.matmul(
                                psx[:],
                                wt[:, cb, ob * P:(ob + 1) * P],
                                xt[:, cb, t2 * FT:(t2 + 1) * FT],
                                start=(ti == 0 and cb == 0),
                                stop=(ti == nt - 1 and cb == CB - 1),
                                skip_group_check=True,
                            )
            state[("ps1", fc)] = ps1

        def emit_eirelu(fc):
            ps1 = state.pop(("ps1", fc))
            ei_t = eipool.tile([P, CB, FC], F32, tag="ei", name=f"ei{fc}")
            for t2 in range(2):
                for ob in range(CB):
                    nc.scalar.activation(
                        ei_t[:, ob, t2 * FT:(t2 + 1) * FT], ps1[(ob, t2)][:],
                        AF.Relu, bias=bei[:, ob:ob + 1])
            state[("ei", fc)] = ei_t

        def emit_split(fc):
            ei_t = state[("ei", fc)]
            eih_t = espool.tile([P, CB, FC], F32R, tag="eih", name=f"eih{fc}")
            eil_t = espool.tile([P, CB, FC], F32R, tag="eil", name=f"eil{fc}")
            for t2 in range(2):
                sl = slice(t2 * FT, (t2 + 1) * FT)
                nc.vector.tensor_scalar(eih_t[:, :, sl], ei_t[:, :, sl], 1.0,
                                        None, op0=OP.mult)
                nc.gpsimd.tensor_tensor(eil_t[:, :, sl], ei_t[:, :, sl],
                                        eih_t[:, :, sl], op=OP.subtract)
            state.pop(("ei", fc))
            state[("eihl", fc)] = (eih_t, eil_t)

        def emit_gemm2(fc):
            eih_t, eil_t = state.pop(("eihl", fc))
            ps2 = {}
            for t2 in range(2):
                for ob in range(CB):
                    psx = wpool.tile([P, FT], F32, tag=f"g{ob}{t2}", name=f"ps2_{fc}_{ob}{t2}")
                    ps2[(ob, t2)] = psx
                    terms = [(waTh, eih_t), (waTl, eih_t), (waTh, eil_t)]
                    nt = len(terms)
                    for ti, (wt, et) in enumerate(terms):
                        for cb in range(CB):
                            nc.tensor.matmul(
                                psx[:],
                                wt[:, cb, ob * P:(ob + 1) * P],
                                et[:, cb, t2 * FT:(t2 + 1) * FT],
                                start=(ti == 0 and cb == 0),
                                stop=(ti == nt - 1 and cb == CB - 1),
                                skip_group_check=True,
                            )
            state[("ps2", fc)] = ps2

        def emit_acopy(fc):
            ps2 = state.pop(("ps2", fc))
            A_t = apool.tile([P, CB, FC], F32, tag="A", name=f"A{fc}")
            for t2 in range(2):
                for ob in range(CB):
                    dst = A_t[:, ob, t2 * FT:(t2 + 1) * FT]
                    nc.vector.tensor_scalar(dst, ps2[(ob, t2)][:], 1.0,
                                            None, op0=OP.mult)
            state[("A", fc)] = A_t

        # ---- prologue: chunk 0 GEMM pipeline ------------------------------------
        emit_xdma(0)
        waTh = load(rr(waTh_d), [P, CB, C], "waTh", F32R)
        waTl = load(rr(waTl_d), [P, CB, C], "waTl", F32R)
        exT = load_act(exT_d[:, :].rearrange("(cb p) n -> p cb n", p=P), [P, CB, N], "exT")
        wexT = load_act(rr(wexT_d), [P, CB, C], "wexT")
        bex = load_act(bex_d[:, :], [P, CB], "bex")
        wbT = load_act(rr(wbT_d), [P, CB, C], "wbT")
        emit_gemm1(0)
        emit_eirelu(0)
        emit_split(0)

        # ---- exemplar branch: ee = relu(WexT.T @ exT + bex);
        #      Dt = WbT.T @ ee + bA (bn2 bias folded into every column) -------
        ee = singles.tile([P, CB, N], F32)
        eeps = wpool.tile([P, FT], F32, tag="g00", name="ee_ps")
        for ob in range(CB):
            for cb in range(CB):
                nc.tensor.matmul(
                    eeps[:, ob * N:ob * N + N],
                    wexT[:, cb, ob * P:(ob + 1) * P],
                    exT[:, cb, :],
                    start=(cb == 0 and ob == 0), stop=(cb == CB - 1 and ob == CB - 1),
                    skip_group_check=True,
                )
        for ob in range(CB):
            nc.scalar.activation(ee[:, ob, :], eeps[:, ob * N:ob * N + N],
                                 AF.Relu, bias=bex[:, ob:ob + 1])
        Dt = singles.tile([P, CB, N], F32)
        dps = wpool.tile([P, FT], F32, tag="g00", name="d_ps")
        for ob in range(CB):
            for eb in range(CB):
                nc.tensor.matmul(
                    dps[:, ob * N:ob * N + N],
                    wbT[:, eb, ob * P:(ob + 1) * P],
                    ee[:, eb, :],
                    start=(eb == 0 and ob == 0), stop=(eb == CB - 1 and ob == CB - 1),
                    skip_group_check=True,
                )
        for ob in range(CB):
            nc.scalar.activation(Dt[:, ob, :], dps[:, ob * N:ob * N + N],
                                 AF.Identity, bias=bA[:, ob:ob + 1])

        emit_gemm2(0)
        emit_acopy(0)
        emit_xdma(1)
        ident = load(ident_d[:, :], [P, P], "ident")

        sim_sb = singles.tile([N, HW], F32)
        pmax = singles.tile([N, NCH], F32)   # running (cumulative) chunk maxima
        nmk = singles.tile([N, NCH], F32)    # -10 * running max per chunk
        dens = singles.tile([N, NCH], F32)   # per-chunk exp-sum vs its running max

        # ---- chunk loop ----------------------------------------------------------
        for fc in range(NCH):
            f0 = fc * FC
            A_t = state.pop(("A", fc))
            simT_ps = stpool.tile([P, P], F32, tag="simT", name=f"simT{fc}")
            nxt = fc + 1 if fc + 1 < NCH else None
            if nxt is not None:
                emit_gemm1(nxt)
            for n in range(N):
                r_t = rpool.tile([P, CB, FC], F32, tag="r", name=f"r{fc}_{n}")
                for cb in range(CB):
                    eng = (RELU_ENG[n] if fc < NCH - 1 else RELU_ENG_LAST[n])[cb]
                    if eng == "d":
                        nc.vector.tensor_scalar(
                            r_t[:, cb, :], A_t[:, cb, :], Dt[:, cb, n:n + 1],
                            0.0, op0=OP.add, op1=OP.max)
                    elif eng == "p":
                        nc.gpsimd.tensor_scalar(
                            r_t[:, cb, :], A_t[:, cb, :], Dt[:, cb, n:n + 1],
                            0.0, op0=OP.add, op1=OP.max)
                    else:
                        nc.scalar.activation(
                            r_t[:, cb, :], A_t[:, cb, :], AF.Relu,
                            bias=Dt[:, cb, n:n + 1])
                sq_t = sqpool.tile([P, CB, FC], F32, tag="sq", name=f"sq{fc}_{n}")
                seng = SQ_ENG[n] if fc < NCH - 1 else SQ_ENG_LAST[n]
                if seng == "a":
                    nc.scalar.activation(sq_t[:], r_t[:], AF.Square)
                elif seng == "p":
                    nc.gpsimd.tensor_mul(sq_t[:], r_t[:], r_t[:])
                elif seng == "s":
                    nc.scalar.activation(sq_t[:, 0, :], r_t[:, 0, :], AF.Square)
                    nc.gpsimd.tensor_mul(sq_t[:, 1, :], r_t[:, 1, :], r_t[:, 1, :])
                else:
                    nc.vector.scalar_tensor_tensor(
                        sq_t[:], r_t[:], 1.0, r_t[:], op0=OP.mult, op1=OP.mult)
                for cb in range(CB):
                    for b in range(NBLK):
                        nc.tensor.matmul(
                            simT_ps[:, b * N:(b + 1) * N],
                            sq_t[:, cb, b * P:(b + 1) * P],
                            zsel[:, N - 1 - n:2 * N - 1 - n],
                            start=(n == 0 and cb == 0 and b == 0),
                            stop=(n == N - 1 and cb == CB - 1 and b == NBLK - 1),
                            skip_group_check=True,
                        )
                # chunk fc+1 GEMM pipeline, interleaved into this chunk's queues
                if nxt is not None:
                    if n == 0 and nxt + 1 < NCH:
                        emit_xdma(nxt + 1)
                    elif n == 5:
                        emit_eirelu(nxt)
                    elif n == 7:
                        emit_split(nxt)
                    elif n == 9:
                        emit_gemm2(nxt)
                    elif n == 13:
                        emit_acopy(nxt)

            # evacuate sim^T, transpose back to [16, f], then chunk softmax piece:
            # exp against the RUNNING max M_fc = max(pmax[0..fc]); denominators
            # are rescaled at the end by gamma_fc = exp(10*(M_fc - M)) <= 1.
            simT_sb = stspool.tile([P, P], F32, tag="simTsb", name=f"simTsb{fc}")
            nc.vector.tensor_scalar(simT_sb[:], simT_ps[:], 1.0, None, op0=OP.mult)
            sim_ps = sim_pool.tile([N, FC], F32, tag="sim", name=f"sim_ps{fc}")
            if fc < NCH - 1:
                for b in range(NBLK):
                    nc.tensor.transpose(
                        sim_ps[:, b * P:(b + 1) * P], simT_sb[:, b * N:(b + 1) * N],
                        ident[:])
                nc.vector.reduce_max(pmax[:, fc:fc + 1], sim_ps[:], axis=AX)
            else:
                # split the max into halves pipelined with the transposes to
                # shorten the tail's serial chain
                pmh = singles.tile([N, 2], F32)
                for half in range(2):
                    for b in range(half * NBLK // 2, (half + 1) * NBLK // 2):
                        nc.tensor.transpose(
                            sim_ps[:, b * P:(b + 1) * P],
                            simT_sb[:, b * N:(b + 1) * N], ident[:])
                    nc.vector.reduce_max(
                        pmh[:, half:half + 1],
                        sim_ps[:, half * FC // 2:(half + 1) * FC // 2], axis=AX)
                nc.vector.reduce_max(pmax[:, fc:fc + 1], pmh[:], axis=AX)
            if fc > 0:
                nc.vector.tensor_tensor(pmax[:, fc:fc + 1], pmax[:, fc:fc + 1],
                                        pmax[:, fc - 1:fc], op=OP.max)
            nc.vector.tensor_scalar_mul(nmk[:, fc:fc + 1], pmax[:, fc:fc + 1],
                                        -INV_TEMP)
            nc.scalar.activation(
                sim_sb[:, f0:f0 + FC], sim_ps[:],
                AF.Exp, bias=nmk[:, fc:fc + 1], scale=INV_TEMP,
                accum_out=dens[:, fc:fc + 1],
            )

        # ---- softmax epilogue ----------------------------------------------------
        # M = pmax[:, NCH-1]; gamma_k = exp(10*(M_k - M)); den = sum_k gam_k*dens_k
        nmx = singles.tile([N, 1], F32)
        nc.vector.tensor_scalar_mul(nmx[:], pmax[:, NCH - 1:NCH], -INV_TEMP)
        gam = singles.tile([N, NCH], F32)
        nc.scalar.activation(gam[:], pmax[:], AF.Exp, bias=nmx[:], scale=INV_TEMP)
        gd = singles.tile([N, NCH], F32)
        nc.vector.tensor_mul(gd[:], gam[:], dens[:])
        den = singles.tile([N, 1], F32)
        nc.vector.reduce_sum(den[:], gd[:], axis=AX)
        rden = singles.tile([N, 1], F32)
        nc.vector.reciprocal(rden[:], den[:])
        grden = singles.tile([N, NCH], F32)
        nc.vector.tensor_scalar(grden[:], gam[:], rden[:, 0:1], None, op0=OP.mult)
        # normalize chunk k by gam_k/den; norms and stores spread across queues
        nc.vector.tensor_scalar_mul(sim_sb[:, 0:FC], sim_sb[:, 0:FC], grden[:, 0:1])
        nc.sync.dma_start(out_d[:, 0:FC], sim_sb[:, 0:FC])
        nc.gpsimd.tensor_scalar(sim_sb[:, FC:2 * FC], sim_sb[:, FC:2 * FC],
                                grden[:, 1:2], None, op0=OP.mult)
        nc.scalar.dma_start(out_d[:, FC:2 * FC], sim_sb[:, FC:2 * FC])
        nc.vector.tensor_scalar_mul(sim_sb[:, 2 * FC:3 * FC], sim_sb[:, 2 * FC:3 * FC],
                                    grden[:, 2:3])
        nc.sync.dma_start(out_d[:, 2 * FC:3 * FC], sim_sb[:, 2 * FC:3 * FC])
        nc.gpsimd.tensor_scalar(sim_sb[:, 3 * FC:], sim_sb[:, 3 * FC:],
                                grden[:, 3:4], None, op0=OP.mult)
        nc.scalar.dma_start(out_d[:, 3 * FC:], sim_sb[:, 3 * FC:])

    nc.compile()
    return nc


_NC_CACHE = {}


def _get_nc():
    if "nc" not in _NC_CACHE:
        _NC_CACHE["nc"] = _build_nc()
    return _NC_CACHE["nc"]


def _tf32(x):
    u = np.ascontiguousarray(x, dtype=np.float32).view(np.uint32)
    return ((u + np.uint32(0x1000)) & np.uint32(0xFFFFE000)).view(np.float32)


def _make_in_maps(inputs):
    f32 = np.float32
    img = np.ascontiguousarray(inputs["image_features"], dtype=f32)     # [B,C,H,W]
    ex = np.ascontiguousarray(inputs["exemplar_features"], dtype=f32)   # [B,N,C]

    s1 = (inputs["bn1_gamma"] / np.sqrt(inputs["bn1_var"] + EPS)).astype(f32)
    t1 = (inputs["bn1_beta"] - inputs["bn1_mean"] * s1).astype(f32)
    s2 = (inputs["bn2_gamma"] / np.sqrt(inputs["bn2_var"] + EPS)).astype(f32)
    t2 = (inputs["bn2_beta"] - inputs["bn2_mean"] * s2).astype(f32)

    W_img = np.asarray(inputs["W_img"], f32)
    W_dr = np.asarray(inputs["W_dr"], f32)
    W_ex = np.asarray(inputs["W_ex"], f32)

    wimg_f = s1[:, None] * W_img                       # [o, c]
    bei_full = (s1 * np.asarray(inputs["b_img"], f32) + t1).astype(f32)
    wa_f = s2[:, None] * W_dr[:, :C]
    bA_full = (s2 * np.asarray(inputs["b_dr"], f32) + t2).astype(f32)
    wb_f = s2[:, None] * W_dr[:, C:]
    bex_full = np.asarray(inputs["b_ex"], f32)

    def t(w):  # [o, c] -> [c, o], contiguous
        return np.ascontiguousarray(w.T.astype(f32))

    def pack_bias(v):  # [C] -> [P, CB], v[cb*P + p] at [p, cb]
        return np.ascontiguousarray(v.reshape(CB, P).T.astype(f32))

    def hl(w):  # hi/lo tf32 split
        h = _tf32(w)
        l = _tf32((w - h).astype(f32))
        return h, l

    wimgT = t(wimg_f)
    waT = t(wa_f)
    wimgTh, wimgTl = hl(wimgT)
    waTh, waTl = hl(waT)

    shared = {
        "wimgTh": wimgTh, "wimgTl": wimgTl,
        "waTh": waTh, "waTl": waTl,
        "wexT": t(W_ex),
        "wbT": t(wb_f),
        "bei": pack_bias(bei_full),
        "bA": pack_bias(bA_full),
        "bex": pack_bias(bex_full),
        "ident": np.eye(P, dtype=f32),
    }
    in_maps = []
    for b in range(B):
        m = dict(shared)
        x = np.ascontiguousarray(img[b].reshape(C, HW))
        xh = _tf32(x)
        xl = _tf32((x - xh).astype(f32))
        m["xh"] = xh
        m["xl"] = xl
        m["exT"] = np.ascontiguousarray(ex[b].T.astype(f32))
        in_maps.append(m)
    return in_maps


def _run(inputs, **kw):
    nc = _get_nc()
    in_maps = _make_in_maps(inputs)
    res = run_bass_kernel_spmd(nc, in_maps, core_ids=list(range(B)), **kw)
    out = np.stack([res.results[i]["out"] for i in range(B)])
    return out.reshape(B, N, H, W).astype(np.float32), res


def kernel(**inputs):
    out, _ = _run(inputs)
    return out


# revision 26
# speedup vs baseline: 1.0091x; 1.0024x over previous
"""Trainium2 Bass kernel for ExemplarImageMatching.

Math (per batch b):
  ei  = relu(bn1(W_img @ x))            x = image[b] as [C, HW]
  A   = s2*(Wa @ ei)                    (bn2 scale folded; Wa = W_dr[:, :C])
  ee  = relu(W_ex @ ex_b^T + b_ex)
  D   = s2*(Wb @ ee) + (s2*b_dr + t2)   (bias folded into D columns)
  sim[n, f] = sum_c relu(A[c, f] + D[c, n])^2
  out = softmax(sim / 0.1, axis=f)

Sharding: data-parallel over B across the 8 cores (B == 8), one image per
core; the N loop runs on-core.

Key structure (vs the one-hot fp32 row-sum baseline):
 - GEMM1/GEMM2 are 3-term Karatsuba-style f32r (tf32) matmuls (measured
   end-to-end error ~5e-5; any 2-term variant is ~2e-2).  x is split hi/lo
   on host; ei is split on device (DVE tensor_scalar copy rounds to tf32,
   gpsimd subtract forms the residual).
 - The channel sum runs on the PE with the roles swapped: the squared tile
   sq [128c, 128f] is the STATIONARY operand and a 16-wide one-hot column
   set (zsel slice) is the MOVING operand, accumulating sim^T [128f, 16n]
   slices in PSUM over (n, cb).  The fp32 4-cycle/row penalty then applies
   to a free dim of 16 instead of 512, and the reduction stays exact fp32.
   Per chunk that is 256 tiny matmuls (~27ns each) into one [128, 128]
   PSUM tile, then 8 PE transposes (via identity) restore sim [16, f].
 - Elementwise work is statically balanced across DVE/ACT/Pool:
   relu-adds mostly on DVE (tensor_scalar add+max runs in the 2x_2p DVE
   perf mode: all-SBUF fp32 at half cycle), squares on ACT (Square) and
   Pool (tensor_mul), PSUM evacuations (ei relu, A copy, sim^T copy) on
   Pool which pays no PSUM access penalty.
 - The GEMM pipeline for chunk k+1 (DMA, GEMM1, ei relu, hi/lo split,
   GEMM2, A copy) is emitted at fixed points inside chunk k's n-loop so
   every engine queue stays busy; 4 PSUM banks are reused
   GEMM1(k)->GEMM2(k)->GEMM1(k+1) with A evacuated to SBUF each chunk.

Softmax: per-chunk partial maxima accumulate off the critical path.
Chunks 0..NCH-2 exponentiate against the provisional max M3 while the last
chunk is still computing; a scalar factor gamma = exp(10*(M3 - M)) corrects
their denominators, and per-chunk normalize+DMA pipelines the stores.
"""

from contextlib import ExitStack

import numpy as np

import concourse.bass as bass
import concourse.bacc as bacc
import concourse.tile as tile
from concourse import mybir
from concourse.bass_utils import run_bass_kernel_spmd

B, N, C, H, W = 8, 16, 256, 64, 64
HW = H * W
P = 128
CB = C // P            # channel blocks (2)
FT = 512               # matmul free-dim tile (one PSUM bank of fp32)
FC = 1024              # f-chunk for the big elementwise pass
NCH = HW // FC         # 4
NBLK = FC // P         # 8 f-blocks of 128 per chunk
EPS = 1e-5
INV_TEMP = 10.0

F32 = mybir.dt.float32
F32R = mybir.dt.float32r
AF = mybir.ActivationFunctionType
OP = mybir.AluOpType
AX = mybir.AxisListType.X

# Static engine schedule for the big pass, per n of each chunk.
# relu-add (2 ops of [128,1024]): 'd'=DVE (593ns, 2x mode), 'p'=Pool (853),
#   'a'=ACT (1038).  square (1 op of [128,2048]): 'a'=ACT (1892),
#   'p'=Pool (1707), 'd'=DVE (2193, no 2x for scalar_tensor_tensor).
# GPSIMD cannot touch PSUM on TRN2, so all PSUM evacuations go to ACT/DVE.
RELU_ENG = [("d", "d")] * 16
for _i in (3, 7, 11, 15):
    RELU_ENG[_i] = ("d", "p")
for _i in (5, 13):
    RELU_ENG[_i] = ("d", "a")
SQ_ENG = ["a", "p", "a", "p", "a", "p", "a", "p",
          "a", "p", "a", "p", "a", "p", "p", "p"]
# Last chunk: no successor-chunk work is interleaved, so the balance shifts:
# ACT squares run early (it finishes with exp+norm), Pool takes the late ones.
SQ_ENG_LAST = ["a", "p", "a", "p", "a", "p", "a", "p",
               "a", "p", "p", "p", "s", "s", "s", "s"]
RELU_ENG_LAST = [("d", "d")] * 16
for _i in (1, 4, 7):
    RELU_ENG_LAST[_i] = ("d", "p")
for _i in (6,):
    RELU_ENG_LAST[_i] = ("d", "a")


def _build_nc():
    nc = bacc.Bacc()

    xh_d = nc.dram_tensor("xh", [C, HW], F32R, kind="ExternalInput")
    xl_d = nc.dram_tensor("xl", [C, HW], F32R, kind="ExternalInput")
    wimgTh_d = nc.dram_tensor("wimgTh", [C, C], F32R, kind="ExternalInput")
    wimgTl_d = nc.dram_tensor("wimgTl", [C, C], F32R, kind="ExternalInput")
    waTh_d = nc.dram_tensor("waTh", [C, C], F32R, kind="ExternalInput")
    waTl_d = nc.dram_tensor("waTl", [C, C], F32R, kind="ExternalInput")
    wexT_d = nc.dram_tensor("wexT", [C, C], F32, kind="ExternalInput")
    wbT_d = nc.dram_tensor("wbT", [C, C], F32, kind="ExternalInput")
    exT_d = nc.dram_tensor("exT", [C, N], F32, kind="ExternalInput")
    bei_d = nc.dram_tensor("bei", [P, CB], F32, kind="ExternalInput")
    bA_d = nc.dram_tensor("bA", [P, CB], F32, kind="ExternalInput")
    bex_d = nc.dram_tensor("bex", [P, CB], F32, kind="ExternalInput")
    ident_d = nc.dram_tensor("ident", [P, P], F32, kind="ExternalInput")
    out_d = nc.dram_tensor("out", [N, HW], F32, kind="ExternalOutput")

    with ExitStack() as ctx:
        tc = ctx.enter_context(tile.TileContext(nc))
        singles = ctx.enter_context(tc.tile_pool(name="singles", bufs=1))
        xpool = ctx.enter_context(tc.tile_pool(name="xpool", bufs=2))
        eipool = ctx.enter_context(tc.tile_pool(name="eipool", bufs=1))
        espool = ctx.enter_context(tc.tile_pool(name="espool", bufs=1))
        apool = ctx.enter_context(tc.tile_pool(name="apool", bufs=2))
        rpool = ctx.enter_context(tc.tile_pool(name="rpool", bufs=4))
        sqpool = ctx.enter_context(tc.tile_pool(name="sqpool", bufs=4))
        stspool = ctx.enter_context(tc.tile_pool(name="stspool", bufs=2))
        wpool = ctx.enter_context(tc.tile_pool(name="wps", bufs=1, space="PSUM"))
        stpool = ctx.enter_context(tc.tile_pool(name="stps", bufs=2, space="PSUM"))
        sim_pool = ctx.enter_context(tc.tile_pool(name="sim_ps", bufs=1, space="PSUM"))

        # ---- constants / weights -------------------------------------------------
        # DMA order is latency-critical: chunk-0 x and the GEMM1 weights first
        # (everything funnels through the single SP HWDGE queue), the
        # exemplar-branch weights and ident later.
        def load(dram_ap, shape, tag, dt=F32):
            t = singles.tile(shape, dt, tag=tag, name=tag)
            nc.sync.dma_start(t[:], dram_ap)
            return t

        rr = lambda d: d[:, :].rearrange("(cb p) o -> p cb o", p=P)

        # warmup scratch (memset before anything else; used to ramp the PE
        # clock while the first DMAs are in flight)
        scratch = singles.tile([P, FT], F32)
        nc.gpsimd.memset(scratch[:], 0.0)
        # Z[:, N-1] = 1, rest 0.  Z[:, N-1-n : 2N-1-n] is a [P, N] matrix whose
        # column n is all-ones.
        zsel = singles.tile([P, 2 * N - 1], F32)
        nc.vector.memset(zsel[:], 0.0)
        nc.vector.memset(zsel[:, N - 1:N], 1.0)

        wps = wpool.tile([P, FT], F32, tag="g00", name="warm_ps")
        for i in range(2):
            nc.tensor.matmul(wps[:N, :], zsel[:, :N], scratch[:],
                             start=(i == 0), stop=(i == 1), skip_group_check=True)

        def load_act(dram_ap, shape, tag, dt=F32):
            t = singles.tile(shape, dt, tag=tag, name=tag)
            nc.scalar.dma_start(t[:], dram_ap)
            return t

        wimgTh = load(rr(wimgTh_d), [P, CB, C], "wimgTh", F32R)
        wimgTl = load(rr(wimgTl_d), [P, CB, C], "wimgTl", F32R)
        # constants ride the (otherwise idle) ACT HWDGE queue so the SP queue
        # carries only the GEMM-critical stream
        bei = load_act(bei_d[:, :], [P, CB], "bei")
        bA = load_act(bA_d[:, :], [P, CB], "bA")

        # ---- pipelined GEMM stages (chunk fc), emitted inside chunk fc-1 ---------
        xh_r = xh_d[:, :].rearrange("(cb p) hw -> p cb hw", p=P)
        xl_r = xl_d[:, :].rearrange("(cb p) hw -> p cb hw", p=P)
        state = {}

        def emit_xdma(fc):
            f0 = fc * FC
            xh_t = xpool.tile([P, CB, FC], F32R, tag="xh", name=f"xh{fc}")
            xl_t = xpool.tile([P, CB, FC], F32R, tag="xl", name=f"xl{fc}")
            for t2 in range(2):
                sl_s = slice(t2 * FT, (t2 + 1) * FT)
                sl_d = slice(f0 + t2 * FT, f0 + (t2 + 1) * FT)
                for cb in range(CB):
                    nc.sync.dma_start(xh_t[:, cb, sl_s], xh_r[:, cb, sl_d])
                for cb in range(CB):
                    nc.sync.dma_start(xl_t[:, cb, sl_s], xl_r[:, cb, sl_d])
            state[("x", fc)] = (xh_t, xl_t)

        def emit_gemm1(fc):
            xh_t, xl_t = state.pop(("x", fc))
            ps1 = {}
            for t2 in range(2):
                for ob in range(CB):
                    psx = wpool.tile([P, FT], F32, tag=f"g{ob}{t2}", name=f"ps1_{fc}_{ob}{t2}")
                    ps1[(ob, t2)] = psx
                    terms = [(wimgTh, xh_t), (wimgTl, xh_t), (wimgTh, xl_t)]
                    nt = len(terms)
                    for ti, (wt, xt) in enumerate(terms):
                        for cb in range(CB):
                            nc.tensor.matmul(
                                psx[:],
                                wt[:, cb, ob * P:(ob + 1) * P],
                                xt[:, cb, t2 * FT:(t2 + 1) * FT],
                                start=(ti == 0 and cb == 0),
                                stop=(ti == nt - 1 and cb == CB - 1),
                                skip_group_check=True,
                            )
            state[("ps1", fc)] = ps1

        def emit_eirelu(fc):
            ps1 = state.pop(("ps1", fc))
            ei_t = eipool.tile([P, CB, FC], F32, tag="ei", name=f"ei{fc}")
            for t2 in range(2):
                for ob in range(CB):
                    nc.scalar.activation(
                        ei_t[:, ob, t2 * FT:(t2 + 1) * FT], ps1[(ob, t2)][:],
                        AF.Relu, bias=bei[:, ob:ob + 1])
            state[("ei", fc)] = ei_t

        def emit_split(fc):
            ei_t = state[("ei", fc)]
            eih_t = espool.tile([P, CB, FC], F32R, tag="eih", name=f"eih{fc}")
            eil_t = espool.tile([P, CB, FC], F32R, tag="eil", name=f"eil{fc}")
            for t2 in range(2):
                sl = slice(t2 * FT, (t2 + 1) * FT)
                nc.vector.tensor_scalar(eih_t[:, :, sl], ei_t[:, :, sl], 1.0,
                                        None, op0=OP.mult)
                nc.gpsimd.tensor_tensor(eil_t[:, :, sl], ei_t[:, :, sl],
                                        eih_t[:, :, sl], op=OP.subtract)
            state.pop(("ei", fc))
            state[("eihl", fc)] = (eih_t, eil_t)

        def emit_gemm2(fc):
            eih_t, eil_t = state.pop(("eihl", fc))
            ps2 = {}
            for t2 in range(2):
                for ob in range(CB):
                    psx = wpool.tile([P, FT], F32, tag=f"g{ob}{t2}", name=f"ps2_{fc}_{ob}{t2}")
                    ps2[(ob, t2)] = psx
                    terms = [(waTh, eih_t), (waTl, eih_t), (waTh, eil_t)]
                    nt = len(terms)
                    for ti, (wt, et) in enumerate(terms):
                        for cb in range(CB):
                            nc.tensor.matmul(
                                psx[:],
                                wt[:, cb, ob * P:(ob + 1) * P],
                                et[:, cb, t2 * FT:(t2 + 1) * FT],
                                start=(ti == 0 and cb == 0),
                                stop=(ti == nt - 1 and cb == CB - 1),
                                skip_group_check=True,
                            )
            state[("ps2", fc)] = ps2

        def emit_acopy(fc):
            ps2 = state.pop(("ps2", fc))
            A_t = apool.tile([P, CB, FC], F32, tag="A", name=f"A{fc}")
            for t2 in range(2):
                for ob in range(CB):
                    dst = A_t[:, ob, t2 * FT:(t2 + 1) * FT]
                    if t2 == 1 and ob == 1:
                        nc.scalar.copy(dst, ps2[(ob, t2)][:])
                    else:
                        nc.vector.tensor_scalar(dst, ps2[(ob, t2)][:], 1.0,
                                                None, op0=OP.mult)
            state[("A", fc)] = A_t

        # ---- prologue: chunk 0 GEMM pipeline ------------------------------------
        emit_xdma(0)
        waTh = load(rr(waTh_d), [P, CB, C], "waTh", F32R)
        waTl = load(rr(waTl_d), [P, CB, C], "waTl", F32R)
        exT = load_act(exT_d[:, :].rearrange("(cb p) n -> p cb n", p=P), [P, CB, N], "exT")
        wexT = load_act(rr(wexT_d), [P, CB, C], "wexT")
        bex = load_act(bex_d[:, :], [P, CB], "bex")
        wbT = load_act(rr(wbT_d), [P, CB, C], "wbT")
        emit_gemm1(0)
        emit_eirelu(0)
        emit_split(0)

        # ---- exemplar branch: ee = relu(WexT.T @ exT + bex);
        #      Dt = WbT.T @ ee + bA (bn2 bias folded into every column) -------
        ee = singles.tile([P, CB, N], F32)
        eeps = wpool.tile([P, FT], F32, tag="g00", name="ee_ps")
        for ob in range(CB):
            for cb in range(CB):
                nc.tensor.matmul(
                    eeps[:, ob * N:ob * N + N],
                    wexT[:, cb, ob * P:(ob + 1) * P],
                    exT[:, cb, :],
                    start=(cb == 0 and ob == 0), stop=(cb == CB - 1 and ob == CB - 1),
                    skip_group_check=True,
                )
        for ob in range(CB):
            nc.scalar.activation(ee[:, ob, :], eeps[:, ob * N:ob * N + N],
                                 AF.Relu, bias=bex[:, ob:ob + 1])
        Dt = singles.tile([P, CB, N], F32)
        dps = wpool.tile([P, FT], F32, tag="g00", name="d_ps")
        for ob in range(CB):
            for eb in range(CB):
                nc.tensor.matmul(
                    dps[:, ob * N:ob * N + N],
                    wbT[:, eb, ob * P:(ob + 1) * P],
                    ee[:, eb, :],
                    start=(eb == 0 and ob == 0), stop=(eb == CB - 1 and ob == CB - 1),
                    skip_group_check=True,
                )
        for ob in range(CB):
            nc.scalar.activation(Dt[:, ob, :], dps[:, ob * N:ob * N + N],
                                 AF.Identity, bias=bA[:, ob:ob + 1])

        emit_gemm2(0)
        emit_acopy(0)
        emit_xdma(1)
        ident = load(ident_d[:, :], [P, P], "ident")

        sim_sb = singles.tile([N, HW], F32)
        pmax = singles.tile([N, NCH], F32)   # running (cumulative) chunk maxima
        nmk = singles.tile([N, NCH], F32)    # -10 * running max per chunk
        dens = singles.tile([N, NCH], F32)   # per-chunk exp-sum vs its running max

        # ---- chunk loop ----------------------------------------------------------
        for fc in range(NCH):
            f0 = fc * FC
            A_t = state.pop(("A", fc))
            simT_ps = stpool.tile([P, P], F32, tag="simT", name=f"simT{fc}")
            nxt = fc + 1 if fc + 1 < NCH else None
            if nxt is not None:
                emit_gemm1(nxt)
            for n in range(N):
                r_t = rpool.tile([P, CB, FC], F32, tag="r", name=f"r{fc}_{n}")
                for cb in range(CB):
                    eng = (RELU_ENG[n] if fc < NCH - 1 else RELU_ENG_LAST[n])[cb]
                    if eng == "d":
                        nc.vector.tensor_scalar(
                            r_t[:, cb, :], A_t[:, cb, :], Dt[:, cb, n:n + 1],
                            0.0, op0=OP.add, op1=OP.max)
                    elif eng == "p":
                        nc.gpsimd.tensor_scalar(
                            r_t[:, cb, :], A_t[:, cb, :], Dt[:, cb, n:n + 1],
                            0.0, op0=OP.add, op1=OP.max)
                    else:
                        nc.scalar.activation(
                            r_t[:, cb, :], A_t[:, cb, :], AF.Relu,
                            bias=Dt[:, cb, n:n + 1])
                sq_t = sqpool.tile([P, CB, FC], F32, tag="sq", name=f"sq{fc}_{n}")
                seng = SQ_ENG[n] if fc < NCH - 1 else SQ_ENG_LAST[n]
                if seng == "a":
                    nc.scalar.activation(sq_t[:], r_t[:], AF.Square)
                elif seng == "p":
                    nc.gpsimd.tensor_mul(sq_t[:], r_t[:], r_t[:])
                elif seng == "s":
                    nc.scalar.activation(sq_t[:, 0, :], r_t[:, 0, :], AF.Square)
                    nc.gpsimd.tensor_mul(sq_t[:, 1, :], r_t[:, 1, :], r_t[:, 1, :])
                else:
                    nc.vector.scalar_tensor_tensor(
                        sq_t[:], r_t[:], 1.0, r_t[:], op0=OP.mult, op1=OP.mult)
                for cb in range(CB):
                    for b in range(NBLK):
                        nc.tensor.matmul(
                            simT_ps[:, b * N:(b + 1) * N],
                            sq_t[:, cb, b * P:(b + 1) * P],
                            zsel[:, N - 1 - n:2 * N - 1 - n],
                            start=(n == 0 and cb == 0 and b == 0),
                            stop=(n == N - 1 and cb == CB - 1 and b == NBLK - 1),
                            skip_group_check=True,
                        )
                # chunk fc+1 GEMM pipeline, interleaved into this chunk's queues
                if nxt is not None:
                    if n == 0 and nxt + 1 < NCH:
                        emit_xdma(nxt + 1)
                    elif n == 5:
                        emit_eirelu(nxt)
                    elif n == 7:
                        emit_split(nxt)
                    elif n == 8:
                        emit_gemm2(nxt)
                    elif n == 13:
                        emit_acopy(nxt)

            # evacuate sim^T, transpose back to [16, f], then chunk softmax piece:
            # exp against the RUNNING max M_fc = max(pmax[0..fc]); denominators
            # are rescaled at the end by gamma_fc = exp(10*(M_fc - M)) <= 1.
            simT_sb = stspool.tile([P, P], F32, tag="simTsb", name=f"simTsb{fc}")
            nc.vector.tensor_scalar(simT_sb[:], simT_ps[:], 1.0, None, op0=OP.mult)
            sim_ps = sim_pool.tile([N, FC], F32, tag="sim", name=f"sim_ps{fc}")
            if fc < NCH - 1:
                for b in range(NBLK):
                    nc.tensor.transpose(
                        sim_ps[:, b * P:(b + 1) * P], simT_sb[:, b * N:(b + 1) * N],
                        ident[:])
                nc.vector.reduce_max(pmax[:, fc:fc + 1], sim_ps[:], axis=AX)
            else:
                # split the max into halves pipelined with the transposes to
                # shorten the tail's serial chain
                pmh = singles.tile([N, 2], F32)
                for half in range(2):
                    for b in range(half * NBLK // 2, (half + 1) * NBLK // 2):
                        nc.tensor.transpose(
                            sim_ps[:, b * P:(b + 1) * P],
                            simT_sb[:, b * N:(b + 1) * N], ident[:])
                    nc.vector.reduce_max(
                        pmh[:, half:half + 1],
                        sim_ps[:, half * FC // 2:(half + 1) * FC // 2], axis=AX)
                nc.vector.reduce_max(pmax[:, fc:fc + 1], pmh[:], axis=AX)
            if fc > 0:
                nc.vector.tensor_tensor(pmax[:, fc:fc + 1], pmax[:, fc:fc + 1],
                                        pmax[:, fc - 1:fc], op=OP.max)
            nc.vector.tensor_scalar_mul(nmk[:, fc:fc + 1], pmax[:, fc:fc + 1],
                                        -INV_TEMP)
            nc.scalar.activation(
                sim_sb[:, f0:f0 + FC], sim_ps[:],
                AF.Exp, bias=nmk[:, fc:fc + 1], scale=INV_TEMP,
                accum_out=dens[:, fc:fc + 1],
            )

        # ---- softmax epilogue ----------------------------------------------------
        # M = pmax[:, NCH-1]; gamma_k = exp(10*(M_k - M)); den = sum_k gam_k*dens_k
        nmx = singles.tile([N, 1], F32)
        nc.vector.tensor_scalar_mul(nmx[:], pmax[:, NCH - 1:NCH], -INV_TEMP)
        gam = singles.tile([N, NCH], F32)
        nc.scalar.activation(gam[:], pmax[:], AF.Exp, bias=nmx[:], scale=INV_TEMP)
        gd = singles.tile([N, NCH], F32)
        nc.vector.tensor_mul(gd[:], gam[:], dens[:])
        den = singles.tile([N, 1], F32)
        nc.vector.reduce_sum(den[:], gd[:], axis=AX)
        rden = singles.tile([N, 1], F32)
        nc.vector.reciprocal(rden[:], den[:])
        grden = singles.tile([N, NCH], F32)
        nc.vector.tensor_scalar(grden[:], gam[:], rden[:, 0:1], None, op0=OP.mult)
        # normalize chunk k by gam_k/den; norms and stores spread across queues
        nc.vector.tensor_scalar_mul(sim_sb[:, 0:FC], sim_sb[:, 0:FC], grden[:, 0:1])
        nc.sync.dma_start(out_d[:, 0:FC], sim_sb[:, 0:FC])
        nc.gpsimd.tensor_scalar(sim_sb[:, FC:2 * FC], sim_sb[:, FC:2 * FC],
                                grden[:, 1:2], None, op0=OP.mult)
        nc.scalar.dma_start(out_d[:, FC:2 * FC], sim_sb[:, FC:2 * FC])
        nc.vector.tensor_scalar_mul(sim_sb[:, 2 * FC:3 * FC], sim_sb[:, 2 * FC:3 * FC],
                                    grden[:, 2:3])
        nc.sync.dma_start(out_d[:, 2 * FC:3 * FC], sim_sb[:, 2 * FC:3 * FC])
        nc.gpsimd.tensor_scalar(sim_sb[:, 3 * FC:], sim_sb[:, 3 * FC:],
                                grden[:, 3:4], None, op0=OP.mult)
        nc.scalar.dma_start(out_d[:, 3 * FC:], sim_sb[:, 3 * FC:])

    nc.compile()
    return nc


_NC_CACHE = {}


def _get_nc():
    if "nc" not in _NC_CACHE:
        _NC_CACHE["nc"] = _build_nc()
    return _NC_CACHE["nc"]


def _tf32(x):
    u = np.ascontiguousarray(x, dtype=np.float32).view(np.uint32)
    return ((u + np.uint32(0x1000)) & np.uint32(0xFFFFE000)).view(np.float32)


def _make_in_maps(inputs):
    f32 = np.float32
    img = np.ascontiguousarray(inputs["image_features"], dtype=f32)     # [B,C,H,W]
    ex = np.ascontiguousarray(inputs["exemplar_features"], dtype=f32)   # [B,N,C]

    s1 = (inputs["bn1_gamma"] / np.sqrt(inputs["bn1_var"] + EPS)).astype(f32)
    t1 = (inputs["bn1_beta"] - inputs["bn1_mean"] * s1).astype(f32)
    s2 = (inputs["bn2_gamma"] / np.sqrt(inputs["bn2_var"] + EPS)).astype(f32)
    t2 = (inputs["bn2_beta"] - inputs["bn2_mean"] * s2).astype(f32)

    W_img = np.asarray(inputs["W_img"], f32)
    W_dr = np.asarray(inputs["W_dr"], f32)
    W_ex = np.asarray(inputs["W_ex"], f32)

    wimg_f = s1[:, None] * W_img                       # [o, c]
    bei_full = (s1 * np.asarray(inputs["b_img"], f32) + t1).astype(f32)
    wa_f = s2[:, None] * W_dr[:, :C]
    bA_full = (s2 * np.asarray(inputs["b_dr"], f32) + t2).astype(f32)
    wb_f = s2[:, None] * W_dr[:, C:]
    bex_full = np.asarray(inputs["b_ex"], f32)

    def t(w):  # [o, c] -> [c, o], contiguous
        return np.ascontiguousarray(w.T.astype(f32))

    def pack_bias(v):  # [C] -> [P, CB], v[cb*P + p] at [p, cb]
        return np.ascontiguousarray(v.reshape(CB, P).T.astype(f32))

    def hl(w):  # hi/lo tf32 split
        h = _tf32(w)
        l = _tf32((w - h).astype(f32))
        return h, l

    wimgT = t(wimg_f)
    waT = t(wa_f)
    wimgTh, wimgTl = hl(wimgT)
    waTh, waTl = hl(waT)

    shared = {
        "wimgTh": wimgTh, "wimgTl": wimgTl,
        "waTh": waTh, "waTl": waTl,
        "wexT": t(W_ex),
        "wbT": t(wb_f),
        "bei": pack_bias(bei_full),
        "bA": pack_bias(bA_full),
        "bex": pack_bias(bex_full),
        "ident": np.eye(P, dtype=f32),
    }
    in_maps = []
    for b in range(B):
        m = dict(shared)
        x = np.ascontiguousarray(img[b].reshape(C, HW))
        xh = _tf32(x)
        xl = _tf32((x - xh).astype(f32))
        m["xh"] = xh
        m["xl"] = xl
        m["exT"] = np.ascontiguousarray(ex[b].T.astype(f32))
        in_maps.append(m)
    return in_maps


def _run(inputs, **kw):
    nc = _get_nc()
    in_maps = _make_in_maps(inputs)
    res = run_bass_kernel_spmd(nc, in_maps, core_ids=list(range(B)), **kw)
    out = np.stack([res.results[i]["out"] for i in range(B)])
    return out.reshape(B, N, H, W).astype(np.float32), res


def kernel(**inputs):
    out, _ = _run(inputs)
    return out


# revision 27
# speedup vs baseline: 1.0138x; 1.0047x over previous
"""Trainium2 Bass kernel for ExemplarImageMatching.

Math (per batch b):
  ei  = relu(bn1(W_img @ x))            x = image[b] as [C, HW]
  A   = s2*(Wa @ ei)                    (bn2 scale folded; Wa = W_dr[:, :C])
  ee  = relu(W_ex @ ex_b^T + b_ex)
  D   = s2*(Wb @ ee) + (s2*b_dr + t2)   (bias folded into D columns)
  sim[n, f] = sum_c relu(A[c, f] + D[c, n])^2
  out = softmax(sim / 0.1, axis=f)

Sharding: data-parallel over B across the 8 cores (B == 8), one image per
core; the N loop runs on-core.

Key structure (vs the one-hot fp32 row-sum baseline):
 - GEMM1/GEMM2 are 3-term Karatsuba-style f32r (tf32) matmuls (measured
   end-to-end error ~5e-5; any 2-term variant is ~2e-2).  x is split hi/lo
   on host; ei is split on device (DVE tensor_scalar copy rounds to tf32,
   gpsimd subtract forms the residual).
 - The channel sum runs on the PE with the roles swapped: the squared tile
   sq [128c, 128f] is the STATIONARY operand and a 16-wide one-hot column
   set (zsel slice) is the MOVING operand, accumulating sim^T [128f, 16n]
   slices in PSUM over (n, cb).  The fp32 4-cycle/row penalty then applies
   to a free dim of 16 instead of 512, and the reduction stays exact fp32.
   Per chunk that is 256 tiny matmuls (~27ns each) into one [128, 128]
   PSUM tile, then 8 PE transposes (via identity) restore sim [16, f].
 - Elementwise work is statically balanced across DVE/ACT/Pool:
   relu-adds mostly on DVE (tensor_scalar add+max runs in the 2x_2p DVE
   perf mode: all-SBUF fp32 at half cycle), squares on ACT (Square) and
   Pool (tensor_mul), PSUM evacuations (ei relu, A copy, sim^T copy) on
   Pool which pays no PSUM access penalty.
 - The GEMM pipeline for chunk k+1 (DMA, GEMM1, ei relu, hi/lo split,
   GEMM2, A copy) is emitted at fixed points inside chunk k's n-loop so
   every engine queue stays busy; 4 PSUM banks are reused
   GEMM1(k)->GEMM2(k)->GEMM1(k+1) with A evacuated to SBUF each chunk.

Softmax: per-chunk partial maxima accumulate off the critical path.
Chunks 0..NCH-2 exponentiate against the provisional max M3 while the last
chunk is still computing; a scalar factor gamma = exp(10*(M3 - M)) corrects
their denominators, and per-chunk normalize+DMA pipelines the stores.
"""

from contextlib import ExitStack

import numpy as np

import concourse.bass as bass
import concourse.bacc as bacc
import concourse.tile as tile
from concourse import mybir
from concourse.bass_utils import run_bass_kernel_spmd

B, N, C, H, W = 8, 16, 256, 64, 64
HW = H * W
P = 128
CB = C // P            # channel blocks (2)
FT = 512               # matmul free-dim tile (one PSUM bank of fp32)
FC = 1024              # f-chunk for the big elementwise pass
NCH = HW // FC         # 4
NBLK = FC // P         # 8 f-blocks of 128 per chunk
EPS = 1e-5
INV_TEMP = 10.0

F32 = mybir.dt.float32
F32R = mybir.dt.float32r
AF = mybir.ActivationFunctionType
OP = mybir.AluOpType
AX = mybir.AxisListType.X

# Static engine schedule for the big pass, per n of each chunk.
# relu-add (2 ops of [128,1024]): 'd'=DVE (593ns, 2x mode), 'p'=Pool (853),
#   'a'=ACT (1038).  square (1 op of [128,2048]): 'a'=ACT (1892),
#   'p'=Pool (1707), 'd'=DVE (2193, no 2x for scalar_tensor_tensor).
# GPSIMD cannot touch PSUM on TRN2, so all PSUM evacuations go to ACT/DVE.
RELU_ENG = [("d", "d")] * 16
for _i in (3, 7, 11, 15):
    RELU_ENG[_i] = ("d", "p")
for _i in (5, 13):
    RELU_ENG[_i] = ("d", "a")
SQ_ENG = ["a", "p", "a", "p", "a", "p", "a", "p",
          "a", "p", "a", "p", "a", "p", "p", "p"]
# Last chunk: no successor-chunk work is interleaved, so the balance shifts:
# ACT squares run early (it finishes with exp+norm), Pool takes the late ones.
SQ_ENG_LAST = ["a", "p", "a", "p", "a", "p", "a", "p",
               "a", "p", "p", "p", "s", "s", "s", "s"]
RELU_ENG_LAST = [("d", "d")] * 16
for _i in (1, 4, 7):
    RELU_ENG_LAST[_i] = ("d", "p")
for _i in (6,):
    RELU_ENG_LAST[_i] = ("d", "a")


def _build_nc():
    nc = bacc.Bacc()

    xh_d = nc.dram_tensor("xh", [C, HW], F32R, kind="ExternalInput")
    xl_d = nc.dram_tensor("xl", [C, HW], F32R, kind="ExternalInput")
    wimgTh_d = nc.dram_tensor("wimgTh", [C, C], F32R, kind="ExternalInput")
    wimgTl_d = nc.dram_tensor("wimgTl", [C, C], F32R, kind="ExternalInput")
    waTh_d = nc.dram_tensor("waTh", [C, C], F32R, kind="ExternalInput")
    waTl_d = nc.dram_tensor("waTl", [C, C], F32R, kind="ExternalInput")
    wexT_d = nc.dram_tensor("wexT", [C, C], F32, kind="ExternalInput")
    wbT_d = nc.dram_tensor("wbT", [C, C], F32, kind="ExternalInput")
    exT_d = nc.dram_tensor("exT", [C, N], F32, kind="ExternalInput")
    bei_d = nc.dram_tensor("bei", [P, CB], F32, kind="ExternalInput")
    bA_d = nc.dram_tensor("bA", [P, CB], F32, kind="ExternalInput")
    bex_d = nc.dram_tensor("bex", [P, CB], F32, kind="ExternalInput")
    ident_d = nc.dram_tensor("ident", [P, P], F32, kind="ExternalInput")
    out_d = nc.dram_tensor("out", [N, HW], F32, kind="ExternalOutput")

    with ExitStack() as ctx:
        tc = ctx.enter_context(tile.TileContext(nc))
        singles = ctx.enter_context(tc.tile_pool(name="singles", bufs=1))
        xpool = ctx.enter_context(tc.tile_pool(name="xpool", bufs=2))
        eipool = ctx.enter_context(tc.tile_pool(name="eipool", bufs=1))
        espool = ctx.enter_context(tc.tile_pool(name="espool", bufs=2))
        apool = ctx.enter_context(tc.tile_pool(name="apool", bufs=2))
        rpool = ctx.enter_context(tc.tile_pool(name="rpool", bufs=5))
        sqpool = ctx.enter_context(tc.tile_pool(name="sqpool", bufs=5))
        stspool = ctx.enter_context(tc.tile_pool(name="stspool", bufs=2))
        wpool = ctx.enter_context(tc.tile_pool(name="wps", bufs=1, space="PSUM"))
        stpool = ctx.enter_context(tc.tile_pool(name="stps", bufs=2, space="PSUM"))
        sim_pool = ctx.enter_context(tc.tile_pool(name="sim_ps", bufs=1, space="PSUM"))

        # ---- constants / weights -------------------------------------------------
        # DMA order is latency-critical: chunk-0 x and the GEMM1 weights first
        # (everything funnels through the single SP HWDGE queue), the
        # exemplar-branch weights and ident later.
        def load(dram_ap, shape, tag, dt=F32):
            t = singles.tile(shape, dt, tag=tag, name=tag)
            nc.sync.dma_start(t[:], dram_ap)
            return t

        rr = lambda d: d[:, :].rearrange("(cb p) o -> p cb o", p=P)

        # warmup scratch (memset before anything else; used to ramp the PE
        # clock while the first DMAs are in flight)
        scratch = singles.tile([P, FT], F32)
        nc.gpsimd.memset(scratch[:], 0.0)
        # Z[:, N-1] = 1, rest 0.  Z[:, N-1-n : 2N-1-n] is a [P, N] matrix whose
        # column n is all-ones.
        zsel = singles.tile([P, 2 * N - 1], F32)
        nc.vector.memset(zsel[:], 0.0)
        nc.vector.memset(zsel[:, N - 1:N], 1.0)

        wps = wpool.tile([P, FT], F32, tag="g00", name="warm_ps")
        for i in range(2):
            nc.tensor.matmul(wps[:N, :], zsel[:, :N], scratch[:],
                             start=(i == 0), stop=(i == 1), skip_group_check=True)

        def load_act(dram_ap, shape, tag, dt=F32):
            t = singles.tile(shape, dt, tag=tag, name=tag)
            nc.scalar.dma_start(t[:], dram_ap)
            return t

        wimgTh = load(rr(wimgTh_d), [P, CB, C], "wimgTh", F32R)
        wimgTl = load(rr(wimgTl_d), [P, CB, C], "wimgTl", F32R)
        # constants ride the (otherwise idle) ACT HWDGE queue so the SP queue
        # carries only the GEMM-critical stream
        bei = load_act(bei_d[:, :], [P, CB], "bei")
        bA = load_act(bA_d[:, :], [P, CB], "bA")

        # ---- pipelined GEMM stages (chunk fc), emitted inside chunk fc-1 ---------
        xh_r = xh_d[:, :].rearrange("(cb p) hw -> p cb hw", p=P)
        xl_r = xl_d[:, :].rearrange("(cb p) hw -> p cb hw", p=P)
        state = {}

        def emit_xdma(fc):
            f0 = fc * FC
            xh_t = xpool.tile([P, CB, FC], F32R, tag="xh", name=f"xh{fc}")
            xl_t = xpool.tile([P, CB, FC], F32R, tag="xl", name=f"xl{fc}")
            for t2 in range(2):
                sl_s = slice(t2 * FT, (t2 + 1) * FT)
                sl_d = slice(f0 + t2 * FT, f0 + (t2 + 1) * FT)
                for cb in range(CB):
                    nc.sync.dma_start(xh_t[:, cb, sl_s], xh_r[:, cb, sl_d])
                for cb in range(CB):
                    nc.sync.dma_start(xl_t[:, cb, sl_s], xl_r[:, cb, sl_d])
            state[("x", fc)] = (xh_t, xl_t)

        def emit_gemm1(fc):
            xh_t, xl_t = state.pop(("x", fc))
            ps1 = {}
            for t2 in range(2):
                for ob in range(CB):
                    psx = wpool.tile([P, FT], F32, tag=f"g{ob}{t2}", name=f"ps1_{fc}_{ob}{t2}")
                    ps1[(ob, t2)] = psx
                    terms = [(wimgTh, xh_t), (wimgTl, xh_t), (wimgTh, xl_t)]
                    nt = len(terms)
                    for ti, (wt, xt) in enumerate(terms):
                        for cb in range(CB):
                            nc.tensor.matmul(
                                psx[:],
                                wt[:, cb, ob * P:(ob + 1) * P],
                                xt[:, cb, t2 * FT:(t2 + 1) * FT],
                                start=(ti == 0 and cb == 0),
                                stop=(ti == nt - 1 and cb == CB - 1),
                                skip_group_check=True,
                            )
            state[("ps1", fc)] = ps1

        def emit_eirelu(fc):
            ps1 = state.pop(("ps1", fc))
            ei_t = eipool.tile([P, CB, FC], F32, tag="ei", name=f"ei{fc}")
            for t2 in range(2):
                for ob in range(CB):
                    nc.scalar.activation(
                        ei_t[:, ob, t2 * FT:(t2 + 1) * FT], ps1[(ob, t2)][:],
                        AF.Relu, bias=bei[:, ob:ob + 1])
            state[("ei", fc)] = ei_t

        def emit_split(fc):
            ei_t = state[("ei", fc)]
            eih_t = espool.tile([P, CB, FC], F32R, tag="eih", name=f"eih{fc}")
            eil_t = espool.tile([P, CB, FC], F32R, tag="eil", name=f"eil{fc}")
            for t2 in range(2):
                sl = slice(t2 * FT, (t2 + 1) * FT)
                nc.vector.tensor_scalar(eih_t[:, :, sl], ei_t[:, :, sl], 1.0,
                                        None, op0=OP.mult)
                nc.gpsimd.tensor_tensor(eil_t[:, :, sl], ei_t[:, :, sl],
                                        eih_t[:, :, sl], op=OP.subtract)
            state.pop(("ei", fc))
            state[("eihl", fc)] = (eih_t, eil_t)

        def emit_gemm2(fc):
            eih_t, eil_t = state.pop(("eihl", fc))
            ps2 = {}
            for t2 in range(2):
                for ob in range(CB):
                    psx = wpool.tile([P, FT], F32, tag=f"g{ob}{t2}", name=f"ps2_{fc}_{ob}{t2}")
                    ps2[(ob, t2)] = psx
                    terms = [(waTh, eih_t), (waTl, eih_t), (waTh, eil_t)]
                    nt = len(terms)
                    for ti, (wt, et) in enumerate(terms):
                        for cb in range(CB):
                            nc.tensor.matmul(
                                psx[:],
                                wt[:, cb, ob * P:(ob + 1) * P],
                                et[:, cb, t2 * FT:(t2 + 1) * FT],
                                start=(ti == 0 and cb == 0),
                                stop=(ti == nt - 1 and cb == CB - 1),
                                skip_group_check=True,
                            )
            state[("ps2", fc)] = ps2

        def emit_acopy(fc):
            ps2 = state.pop(("ps2", fc))
            A_t = apool.tile([P, CB, FC], F32, tag="A", name=f"A{fc}")
            for t2 in range(2):
                for ob in range(CB):
                    dst = A_t[:, ob, t2 * FT:(t2 + 1) * FT]
                    if t2 == 1 and ob == 1:
                        nc.scalar.copy(dst, ps2[(ob, t2)][:])
                    else:
                        nc.vector.tensor_scalar(dst, ps2[(ob, t2)][:], 1.0,
                                                None, op0=OP.mult)
            state[("A", fc)] = A_t

        # ---- prologue: chunk 0 GEMM pipeline ------------------------------------
        emit_xdma(0)
        waTh = load(rr(waTh_d), [P, CB, C], "waTh", F32R)
        waTl = load(rr(waTl_d), [P, CB, C], "waTl", F32R)
        exT = load_act(exT_d[:, :].rearrange("(cb p) n -> p cb n", p=P), [P, CB, N], "exT")
        wexT = load_act(rr(wexT_d), [P, CB, C], "wexT")
        bex = load_act(bex_d[:, :], [P, CB], "bex")
        wbT = load_act(rr(wbT_d), [P, CB, C], "wbT")
        emit_gemm1(0)
        emit_eirelu(0)
        emit_split(0)

        # ---- exemplar branch: ee = relu(WexT.T @ exT + bex);
        #      Dt = WbT.T @ ee + bA (bn2 bias folded into every column) -------
        ee = singles.tile([P, CB, N], F32)
        eeps = wpool.tile([P, FT], F32, tag="g00", name="ee_ps")
        for ob in range(CB):
            for cb in range(CB):
                nc.tensor.matmul(
                    eeps[:, ob * N:ob * N + N],
                    wexT[:, cb, ob * P:(ob + 1) * P],
                    exT[:, cb, :],
                    start=(cb == 0 and ob == 0), stop=(cb == CB - 1 and ob == CB - 1),
                    skip_group_check=True,
                )
        for ob in range(CB):
            nc.scalar.activation(ee[:, ob, :], eeps[:, ob * N:ob * N + N],
                                 AF.Relu, bias=bex[:, ob:ob + 1])
        Dt = singles.tile([P, CB, N], F32)
        dps = wpool.tile([P, FT], F32, tag="g00", name="d_ps")
        for ob in range(CB):
            for eb in range(CB):
                nc.tensor.matmul(
                    dps[:, ob * N:ob * N + N],
                    wbT[:, eb, ob * P:(ob + 1) * P],
                    ee[:, eb, :],
                    start=(eb == 0 and ob == 0), stop=(eb == CB - 1 and ob == CB - 1),
                    skip_group_check=True,
                )
        for ob in range(CB):
            nc.scalar.activation(Dt[:, ob, :], dps[:, ob * N:ob * N + N],
                                 AF.Identity, bias=bA[:, ob:ob + 1])

        emit_gemm2(0)
        emit_acopy(0)
        emit_xdma(1)
        ident = load(ident_d[:, :], [P, P], "ident")

        sim_sb = singles.tile([N, HW], F32)
        pmax = singles.tile([N, NCH], F32)   # running (cumulative) chunk maxima
        nmk = singles.tile([N, NCH], F32)    # -10 * running max per chunk
        dens = singles.tile([N, NCH], F32)   # per-chunk exp-sum vs its running max

        # ---- chunk loop ----------------------------------------------------------
        for fc in range(NCH):
            f0 = fc * FC
            A_t = state.pop(("A", fc))
            simT_ps = stpool.tile([P, P], F32, tag="simT", name=f"simT{fc}")
            nxt = fc + 1 if fc + 1 < NCH else None
            if nxt is not None:
                emit_gemm1(nxt)
            for n in range(N):
                r_t = rpool.tile([P, CB, FC], F32, tag="r", name=f"r{fc}_{n}")
                for cb in range(CB):
                    eng = (RELU_ENG[n] if fc < NCH - 1 else RELU_ENG_LAST[n])[cb]
                    if eng == "d":
                        nc.vector.tensor_scalar(
                            r_t[:, cb, :], A_t[:, cb, :], Dt[:, cb, n:n + 1],
                            0.0, op0=OP.add, op1=OP.max)
                    elif eng == "p":
                        nc.gpsimd.tensor_scalar(
                            r_t[:, cb, :], A_t[:, cb, :], Dt[:, cb, n:n + 1],
                            0.0, op0=OP.add, op1=OP.max)
                    else:
                        nc.scalar.activation(
                            r_t[:, cb, :], A_t[:, cb, :], AF.Relu,
                            bias=Dt[:, cb, n:n + 1])
                sq_t = sqpool.tile([P, CB, FC], F32, tag="sq", name=f"sq{fc}_{n}")
                seng = SQ_ENG[n] if fc < NCH - 1 else SQ_ENG_LAST[n]
                if seng == "a":
                    nc.scalar.activation(sq_t[:], r_t[:], AF.Square)
                elif seng == "p":
                    nc.gpsimd.tensor_mul(sq_t[:], r_t[:], r_t[:])
                elif seng == "s":
                    nc.scalar.activation(sq_t[:, 0, :], r_t[:, 0, :], AF.Square)
                    nc.gpsimd.tensor_mul(sq_t[:, 1, :], r_t[:, 1, :], r_t[:, 1, :])
                else:
                    nc.vector.scalar_tensor_tensor(
                        sq_t[:], r_t[:], 1.0, r_t[:], op0=OP.mult, op1=OP.mult)
                for cb in range(CB):
                    for b in range(NBLK):
                        nc.tensor.matmul(
                            simT_ps[:, b * N:(b + 1) * N],
                            sq_t[:, cb, b * P:(b + 1) * P],
                            zsel[:, N - 1 - n:2 * N - 1 - n],
                            start=(n == 0 and cb == 0 and b == 0),
                            stop=(n == N - 1 and cb == CB - 1 and b == NBLK - 1),
                            skip_group_check=True,
                        )
                # chunk fc+1 GEMM pipeline, interleaved into this chunk's queues
                if nxt is not None:
                    if n == 0 and nxt + 1 < NCH:
                        emit_xdma(nxt + 1)
                    elif n == 5:
                        emit_eirelu(nxt)
                    elif n == 7:
                        emit_split(nxt)
                    elif n == 8:
                        emit_gemm2(nxt)
                    elif n == 13:
                        emit_acopy(nxt)

            # evacuate sim^T, transpose back to [16, f], then chunk softmax piece:
            # exp against the RUNNING max M_fc = max(pmax[0..fc]); denominators
            # are rescaled at the end by gamma_fc = exp(10*(M_fc - M)) <= 1.
            simT_sb = stspool.tile([P, P], F32, tag="simTsb", name=f"simTsb{fc}")
            nc.vector.tensor_scalar(simT_sb[:], simT_ps[:], 1.0, None, op0=OP.mult)
            sim_ps = sim_pool.tile([N, FC], F32, tag="sim", name=f"sim_ps{fc}")
            if fc < NCH - 1:
                for b in range(NBLK):
                    nc.tensor.transpose(
                        sim_ps[:, b * P:(b + 1) * P], simT_sb[:, b * N:(b + 1) * N],
                        ident[:])
                nc.vector.reduce_max(pmax[:, fc:fc + 1], sim_ps[:], axis=AX)
            else:
                # split the max into halves pipelined with the transposes to
                # shorten the tail's serial chain
                pmh = singles.tile([N, 2], F32)
                for half in range(2):
                    for b in range(half * NBLK // 2, (half + 1) * NBLK // 2):
                        nc.tensor.transpose(
                            sim_ps[:, b * P:(b + 1) * P],
                            simT_sb[:, b * N:(b + 1) * N], ident[:])
                    nc.vector.reduce_max(
                        pmh[:, half:half + 1],
                        sim_ps[:, half * FC // 2:(half + 1) * FC // 2], axis=AX)
                nc.vector.reduce_max(pmax[:, fc:fc + 1], pmh[:], axis=AX)
            if fc > 0:
                nc.vector.tensor_tensor(pmax[:, fc:fc + 1], pmax[:, fc:fc + 1],
                                        pmax[:, fc - 1:fc], op=OP.max)
            nc.vector.tensor_scalar_mul(nmk[:, fc:fc + 1], pmax[:, fc:fc + 1],
                                        -INV_TEMP)
            nc.scalar.activation(
                sim_sb[:, f0:f0 + FC], sim_ps[:],
                AF.Exp, bias=nmk[:, fc:fc + 1], scale=INV_TEMP,
                accum_out=dens[:, fc:fc + 1],
            )

        # ---- softmax epilogue ----------------------------------------------------
        # M = pmax[:, NCH-1]; gamma_k = exp(10*(M_k - M)); den = sum_k gam_k*dens_k
        nmx = singles.tile([N, 1], F32)
        nc.vector.tensor_scalar_mul(nmx[:], pmax[:, NCH - 1:NCH], -INV_TEMP)
        gam = singles.tile([N, NCH], F32)
        nc.scalar.activation(gam[:], pmax[:], AF.Exp, bias=nmx[:], scale=INV_TEMP)
        gd = singles.tile([N, NCH], F32)
        nc.vector.tensor_mul(gd[:], gam[:], dens[:])
        den = singles.tile([N, 1], F32)
        nc.vector.reduce_sum(den[:], gd[:], axis=AX)
        rden = singles.tile([N, 1], F32)
        nc.vector.reciprocal(rden[:], den[:])
        grden = singles.tile([N, NCH], F32)
        nc.vector.tensor_scalar(grden[:], gam[:], rden[:, 0:1], None, op0=OP.mult)
        # normalize chunk k by gam_k/den; norms and stores spread across queues
        nc.vector.tensor_scalar_mul(sim_sb[:, 0:FC], sim_sb[:, 0:FC], grden[:, 0:1])
        nc.sync.dma_start(out_d[:, 0:FC], sim_sb[:, 0:FC])
        nc.gpsimd.tensor_scalar(sim_sb[:, FC:2 * FC], sim_sb[:, FC:2 * FC],
                                grden[:, 1:2], None, op0=OP.mult)
        nc.scalar.dma_start(out_d[:, FC:2 * FC], sim_sb[:, FC:2 * FC])
        nc.vector.tensor_scalar_mul(sim_sb[:, 2 * FC:3 * FC], sim_sb[:, 2 * FC:3 * FC],
                                    grden[:, 2:3])
        nc.sync.dma_start(out_d[:, 2 * FC:3 * FC], sim_sb[:, 2 * FC:3 * FC])
        nc.gpsimd.tensor_scalar(sim_sb[:, 3 * FC:], sim_sb[:, 3 * FC:],
                                grden[:, 3:4], None, op0=OP.mult)
        nc.scalar.dma_start(out_d[:, 3 * FC:], sim_sb[:, 3 * FC:])

    nc.compile()
    return nc


_NC_CACHE = {}


def _get_nc():
    if "nc" not in _NC_CACHE:
        _NC_CACHE["nc"] = _build_nc()
    return _NC_CACHE["nc"]


def _tf32(x):
    u = np.ascontiguousarray(x, dtype=np.float32).view(np.uint32)
    return ((u + np.uint32(0x1000)) & np.uint32(0xFFFFE000)).view(np.float32)


def _make_in_maps(inputs):
    f32 = np.float32
    img = np.ascontiguousarray(inputs["image_features"], dtype=f32)     # [B,C,H,W]
    ex = np.ascontiguousarray(inputs["exemplar_features"], dtype=f32)   # [B,N,C]

    s1 = (inputs["bn1_gamma"] / np.sqrt(inputs["bn1_var"] + EPS)).astype(f32)
    t1 = (inputs["bn1_beta"] - inputs["bn1_mean"] * s1).astype(f32)
    s2 = (inputs["bn2_gamma"] / np.sqrt(inputs["bn2_var"] + EPS)).astype(f32)
    t2 = (inputs["bn2_beta"] - inputs["bn2_mean"] * s2).astype(f32)

    W_img = np.asarray(inputs["W_img"], f32)
    W_dr = np.asarray(inputs["W_dr"], f32)
    W_ex = np.asarray(inputs["W_ex"], f32)

    wimg_f = s1[:, None] * W_img                       # [o, c]
    bei_full = (s1 * np.asarray(inputs["b_img"], f32) + t1).astype(f32)
    wa_f = s2[:, None] * W_dr[:, :C]
    bA_full = (s2 * np.asarray(inputs["b_dr"], f32) + t2).astype(f32)
    wb_f = s2[:, None] * W_dr[:, C:]
    bex_full = np.asarray(inputs["b_ex"], f32)

    def t(w):  # [o, c] -> [c, o], contiguous
        return np.ascontiguousarray(w.T.astype(f32))

    def pack_bias(v):  # [C] -> [P, CB], v[cb*P + p] at [p, cb]
        return np.ascontiguousarray(v.reshape(CB, P).T.astype(f32))

    def hl(w):  # hi/lo tf32 split
        h = _tf32(w)
        l = _tf32((w - h).astype(f32))
        return h, l

    wimgT = t(wimg_f)
    waT = t(wa_f)
    wimgTh, wimgTl = hl(wimgT)
    waTh, waTl = hl(waT)

    shared = {
        "wimgTh": wimgTh, "wimgTl": wimgTl,
        "waTh": waTh, "waTl": waTl,
        "wexT": t(W_ex),
        "wbT": t(wb_f),
        "bei": pack_bias(bei_full),
        "bA": pack_bias(bA_full),
        "bex": pack_bias(bex_full),
        "ident": np.eye(P, dtype=f32),
    }
    in_maps = []
    for b in range(B):
        m = dict(shared)
        x = np.ascontiguousarray(img[b].reshape(C, HW))
        xh = _tf32(x)
        xl = _tf32((x - xh).astype(f32))
        m["xh"] = xh
        m["xl"] = xl
        m["exT"] = np.ascontiguousarray(ex[b].T.astype(f32))
        in_maps.append(m)
    return in_maps


def _run(inputs, **kw):
    nc = _get_nc()
    in_maps = _make_in_maps(inputs)
    res = run_bass_kernel_spmd(nc, in_maps, core_ids=list(range(B)), **kw)
    out = np.stack([res.results[i]["out"] for i in range(B)])
    return out.reshape(B, N, H, W).astype(np.float32), res


def kernel(**inputs):
    out, _ = _run(inputs)
    return out
